# revision 1
# baseline (speedup 1.0000x reference)
"""AttentionPooling (global-softmax segment-sum) Trainium2 Bass kernel.

  scores = x @ W + b ; attn = softmax(scores, axis=0) ; out = segment_sum(x*attn, batch, G)

Design (8 cores, SPMD, raw Bass — no Tile: this walrus build allows only ONE
semaphore wait per instruction, so every cross-engine wait is its own wait_ge):

 * softmax is shift-invariant => b drops out; fixed shift M=0 (scores~N(0,1)).
 * device computes, per core, the unnormalized pooled_g = sum_{i in g} e^{s_i} x_i
   and Z_core = sum_i e^{s_i}; host divides by Z = sum Z_core at the end.
 * shard by SEGMENT BLOCKS: G segs -> cores x blocks x 128 segs. batch is sorted
   => each block's nodes are a contiguous node range; host zero-pads every block
   to one fixed node budget (multiple of 2048) so all 8 cores run the same
   static SPMD program. Pads: x=0 => e^0=1 pollutes Z only (host subtracts the
   pad count); pad batchloc=999 never matches the one-hot => pooled unpolluted.
 * per 4096-node super-chunk s (node n: partition p=n%128, chunk c=n//128):
     SYNC  dma xt[s] <- host-preswizzled bf16 [128, 32x128]
     DVE   xw = xt * Wrep (bf16 TT, 2x mode); tree-fold d 128->8 with 2x TT
           adds (tensor_reduce only runs 1x), then one small 1x reduce ->
           scores[:, 32] f32
     ACT   expw[:,32] = Exp(scores) (accum_out -> Z partial column)
     one-hot per chunk A[p,j] = (iota_j==batchloc_p)*expw_p, split three ways:
       kd chunks on DVE (tensor_scalar is_equal+mult, 4x mode, ~94ns)
       kg chunks on GPSIMD (same op, ~273ns)
       ka chunks on ACT: u=(iota-bl)^2 [Square, bias=-bl]; A=Relu(expw-u*expw)
         [Relu, scale=-expw, bias=+expw] — exact for integer iota/bl.
     PE    psum[128 segs, 128 d] += A.T @ x_chunk (bf16 matmul, 1 cyc/row)
   one-hot work of super s-1 overlaps scores of super s (software pipeline).
 * per 128-seg block: ACT copies psum->sbuf stage; one DMA out at the end.
 * blocks are ragged: blk_ch chunks (e.g. 125 = 32+32+32+29 supers); short
   supers shed DVE one-hot chunks first (DVE is the busiest engine).
 * per-block output slices DMA out as soon as staged (overlaps the tail).
 * TimelineSim (cost model): ~364 us/core; DMA floor ~197 us; engines
   DVE ~340 / Pool ~335 / ACT ~300 / PE ~110 us. Measured full-size
   relative error vs fp32 reference: 0.0059 (bf16 data path).
"""

import os
import numpy as np
import ml_dtypes

import concourse.bass as bass
import concourse.mybir as mybir
from concourse.bass_utils import run_bass_kernel_spmd

BF16 = mybir.dt.bfloat16
F32 = mybir.dt.float32
ALU = mybir.AluOpType
ACTF = mybir.ActivationFunctionType

N_CORES = 8
D = 128
P = 128
SUP_CH = 32            # chunks per super-chunk
SUP = P * SUP_CH       # 2048 nodes per super-chunk
NXB = 12               # x-tile buffer depth (one DMA in flight per slot)
NAT = 10               # one-hot tile slots per producing engine

_prog_cache = {}


def _build(blocks, blk_ch, kd, ka):
    """blocks 128-seg blocks/core; blk_ch = chunks per block (ragged: supers of
    <=SUP_CH chunks); one-hot split per super: kd on DVE, ka on ACT, rest GPSIMD."""
    # per-super chunk counts, uniform across blocks (SPMD)
    sup_shape = []
    r = blk_ch
    while r > 0:
        t = min(SUP_CH, r)
        sup_shape.append(t)
        r -= t
    spb = len(sup_shape)
    nsup = blocks * spb
    ch_of = [sup_shape[s % spb] for s in range(nsup)]
    CH0 = [0]
    for s in range(nsup):
        CH0.append(CH0[-1] + ch_of[s])
    nch = CH0[-1]
    # per-super split (smaller supers shed GPSIMD chunks first, then ACT, DVE)
    kd_of, ka_of, kg_of = [], [], []
    kg_full = SUP_CH - kd - ka
    for s in range(nsup):
        n = ch_of[s]
        # short supers shed DVE chunks first (DVE is the busiest engine)
        g = min(kg_full, n); a = min(ka, n - g); d = n - g - a
        kd_of.append(d); ka_of.append(a); kg_of.append(g)
    nc = bass.Bass()

    xp_h = nc.declare_dram_parameter("xp", [nch * P * D], BF16, isOutput=False)
    bl_h = nc.declare_dram_parameter("bl", [P, nch], F32, isOutput=False)
    wrep_h = nc.declare_dram_parameter("wrep", [P, SUP_CH * D], BF16, isOutput=False)
    iota_h = nc.declare_dram_parameter("iota", [P, P], BF16, isOutput=False)
    bln_h = nc.declare_dram_parameter("bln", [P, nch], F32, isOutput=False)
    out_h = nc.declare_dram_parameter("outp", [P, blocks * D], F32, isOutput=True)
    z_h = nc.declare_dram_parameter("zout", [P, 1], F32, isOutput=True)

    # tick tables (pass 1: pure counting in emission order) -----------------
    # DVE iter s: TT(s)(+1), folds(0), RED(s)(+1) [s<nsup]; then kd_of[s-1] TS (+1 each)
    T_DVE_TT, T_DVE_RED, T_DVE_TS = {}, {}, {}
    t = 0
    for s in range(nsup + 1):
        if s < nsup:
            t += 1; T_DVE_TT[s] = t
            t += 1; T_DVE_RED[s] = t
        if s >= 1:
            for i in range(kd_of[s - 1]):
                t += 1; T_DVE_TS[(s - 1, i)] = t
    zred_tick = t + 1
    # GPSIMD iter s>=1: kg_of[s-1] TS
    T_GP_TS = {}
    t = 0
    for s in range(1, nsup + 1):
        for i in range(kg_of[s - 1]):
            t += 1; T_GP_TS[(s - 1, i)] = t
    # ACT iter s: Exp(+1), negate(0) [s<nsup]; ka_of[s-1] pairs (+1 each on Relu)
    T_ACT_EXP, T_ACT_OH = {}, {}
    t = 0
    for s in range(nsup + 1):
        if s < nsup:
            t += 1; T_ACT_EXP[s] = t
        if s >= 1:
            for j in range(ka_of[s - 1]):
                t += 1; T_ACT_OH[(s - 1, j)] = t
    # PE: one mm per chunk, supers in order
    T_PE_MM = {}
    t = 0
    for s in range(nsup):
        for c in range(ch_of[s]):
            t += 1; T_PE_MM[(s, c)] = t

    def t_dve_tt(s):
        return T_DVE_TT[s]

    def t_dve_red(s):
        return T_DVE_RED[s]

    def t_dve_ts(sm1, i):
        return T_DVE_TS[(sm1, i)]

    def t_gp_ts(sm1, i):
        return T_GP_TS[(sm1, i)]

    def t_act_exp(s):
        return T_ACT_EXP[s]

    def t_act_oh(sm1, j):
        return T_ACT_OH[(sm1, j)]

    def t_pe_mm(s, c):
        return T_PE_MM[(s, c)]

    DVE_LIST = [(s, i) for s in range(nsup) for i in range(kd_of[s])]
    GP_LIST = [(s, kd_of[s] + i) for s in range(nsup) for i in range(kg_of[s])]
    ACT_LIST = [(s, kd_of[s] + kg_of[s] + j) for s in range(nsup)
                for j in range(ka_of[s])]
    DVE_IDX0 = [0]
    for s in range(nsup):
        DVE_IDX0.append(DVE_IDX0[-1] + kd_of[s])
    GP_IDX0 = [0]
    for s in range(nsup):
        GP_IDX0.append(GP_IDX0[-1] + kg_of[s])
    ACT_IDX0 = [0]
    for s in range(nsup):
        ACT_IDX0.append(ACT_IDX0[-1] + ka_of[s])

    import contextlib
    with contextlib.ExitStack() as ctx:
        sem_xc = ctx.enter_context(nc.semaphore("sem_xc"))
        sem_x = [ctx.enter_context(nc.semaphore(f"sem_x{j}")) for j in range(NXB)]
        sem_dve = ctx.enter_context(nc.semaphore("sem_dve"))
        sem_act = ctx.enter_context(nc.semaphore("sem_act"))
        sem_gp = ctx.enter_context(nc.semaphore("sem_gp"))
        sem_pe = ctx.enter_context(nc.semaphore("sem_pe"))
        sem_cp = ctx.enter_context(nc.semaphore("sem_cp"))
        sem_out = ctx.enter_context(nc.semaphore("sem_out"))

        wrep_t = ctx.enter_context(nc.sbuf_tensor([P, SUP_CH * D], BF16))
        iota_t = ctx.enter_context(nc.sbuf_tensor([P, P], BF16))
        bl_t = ctx.enter_context(nc.sbuf_tensor([P, nch], F32))
        xt = [ctx.enter_context(nc.sbuf_tensor(f"xt{j}", [P, SUP_CH * D], BF16))
              for j in range(NXB)]
        xw_t = ctx.enter_context(nc.sbuf_tensor([P, SUP_CH * D], BF16))
        scores_t = ctx.enter_context(nc.sbuf_tensor([P, nch], F32))
        expw_t = ctx.enter_context(nc.sbuf_tensor([P, nch], F32))
        zc_t = ctx.enter_context(nc.sbuf_tensor([P, nsup], F32))
        zsum_t = ctx.enter_context(nc.sbuf_tensor([P, 1], F32))
        stage_t = ctx.enter_context(nc.sbuf_tensor([P, blocks * D], F32))
        atd = [ctx.enter_context(nc.sbuf_tensor(f"atd{j}", [P, P], BF16)) for j in range(NAT)]
        atg = [ctx.enter_context(nc.sbuf_tensor(f"atg{j}", [P, P], BF16)) for j in range(NAT)]
        ata = [ctx.enter_context(nc.sbuf_tensor(f"ata{j}", [P, P], BF16)) for j in range(NAT)]
        uat = ctx.enter_context(nc.sbuf_tensor("uat", [P, P], BF16))
        bln_t = ctx.enter_context(nc.sbuf_tensor("bln_t", [P, nch], F32))
        nexpw_t = ctx.enter_context(nc.sbuf_tensor("nexpw_t", [P, nch], F32))
        pt = [ctx.enter_context(nc.psum_tensor(f"pt{j}", [P, 512], F32)) for j in range(2)]


        with nc.Block() as block:

            @block.sync
            def _(sync):
                sync.dma_start(out=wrep_t[:], in_=wrep_h[:]).then_inc(sem_xc, 16)
                sync.dma_start(out=iota_t[:], in_=iota_h[:]).then_inc(sem_xc, 16)
                sync.dma_start(out=bl_t[:], in_=bl_h[:]).then_inc(sem_xc, 16)
                sync.dma_start(out=bln_t[:], in_=bln_h[:]).then_inc(sem_xc, 16)
                for s in range(nsup):
                    j = s % NXB
                    ch = ch_of[s]
                    if s >= NXB:
                        so = s - NXB  # slot's previous super: consumers done?
                        sync.wait_ge(sem_dve, t_dve_tt(so))
                        sync.wait_ge(sem_pe, t_pe_mm(so, ch_of[so] - 1))
                    sync.dma_start(
                        out=xt[j][:, 0:ch * D].rearrange("p (c d) -> p c d", d=D),
                        in_=xp_h[CH0[s] * P * D:CH0[s + 1] * P * D].rearrange(
                            "(p c d) -> p c d", p=P, d=D),
                    ).then_inc(sem_x[j], 16)
                # outputs: stream each block's slice as soon as it is staged
                for b in range(blocks):
                    sync.wait_ge(sem_cp, b + 1)
                    sync.dma_start(
                        out=out_h[:, b * D:(b + 1) * D],
                        in_=stage_t[:, b * D:(b + 1) * D],
                    ).then_inc(sem_out, 16)
                sync.wait_ge(sem_dve, zred_tick)
                sync.dma_start(out=z_h[:], in_=zsum_t[:]).then_inc(sem_out, 16)
                sync.wait_ge(sem_out, 16 * (blocks + 1))

            @block.vector
            def _(vector):
                vector.wait_ge(sem_xc, 64)
                for s in range(nsup + 1):
                    if s < nsup:
                        j = s % NXB
                        ch = ch_of[s]
                        vector.wait_ge(sem_x[j], 16 * (s // NXB + 1))
                        nc.vector.tensor_tensor(
                            out=xw_t[:, 0:ch * D], in0=xt[j][:, 0:ch * D],
                            in1=wrep_t[:, 0:ch * D], op=ALU.mult
                        ).then_inc(sem_dve, 1)
                        # tree-fold the d-axis 128->8 with 2x-mode TT adds
                        # (tensor_reduce runs at 1x; folds are ~1.6x cheaper),
                        # then one small 1x reduce. bf16 partials cost ~0.5%
                        # extra score error — fine at the 2e-2 scale.
                        xw3 = xw_t[:, 0:ch * D].rearrange("p (c d) -> p c d", d=D)
                        for w in (64, 32, 16, 8):
                            nc.vector.tensor_tensor(
                                out=xw3[:, :, 0:w], in0=xw3[:, :, 0:w],
                                in1=xw3[:, :, w:2 * w], op=ALU.add,
                            )
                        nc.vector.tensor_reduce(
                            out=scores_t[:, CH0[s]:CH0[s + 1]],
                            in_=xw3[:, :, 0:8],
                            axis=mybir.AxisListType.X, op=ALU.add,
                        ).then_inc(sem_dve, 1)
                    if s >= 1 and kd_of[s - 1] > 0:
                        sm1 = s - 1
                        vector.wait_ge(sem_act, t_act_exp(sm1))
                        for i in range(kd_of[sm1]):
                            gd = DVE_IDX0[sm1] + i
                            if gd >= NAT:  # one-hot slot: wait mm that freed it
                                po, io = DVE_LIST[gd - NAT]
                                vector.wait_ge(sem_pe, t_pe_mm(po, io))
                            ca = CH0[sm1] + i
                            nc.vector.tensor_scalar(
                                atd[gd % NAT][:], iota_t[:],
                                bl_t[:, ca:ca + 1], expw_t[:, ca:ca + 1],
                                ALU.is_equal, ALU.mult,
                            ).then_inc(sem_dve, 1)
                # Z final reduction
                vector.wait_ge(sem_act, t_act_exp(nsup - 1))
                nc.vector.tensor_reduce(
                    out=zsum_t[:], in_=zc_t[:],
                    axis=mybir.AxisListType.X, op=ALU.add,
                ).then_inc(sem_dve, 1)

            @block.gpsimd
            def _(gpsimd):
                gpsimd.wait_ge(sem_xc, 64)
                for s in range(1, nsup + 1):
                    sm1 = s - 1
                    if kg_of[sm1] == 0:
                        continue
                    gpsimd.wait_ge(sem_act, t_act_exp(sm1))
                    for i in range(kg_of[sm1]):
                        gg = GP_IDX0[sm1] + i
                        if gg >= NAT:
                            po, co = GP_LIST[gg - NAT]
                            gpsimd.wait_ge(sem_pe, t_pe_mm(po, co))
                        ca = CH0[sm1] + kd_of[sm1] + i
                        nc.gpsimd.tensor_scalar(
                            atg[gg % NAT][:], iota_t[:],
                            bl_t[:, ca:ca + 1], expw_t[:, ca:ca + 1],
                            ALU.is_equal, ALU.mult,
                        ).then_inc(sem_gp, 1)

            @block.scalar
            def _(scalar):
                scalar.wait_ge(sem_xc, 64)
                for s in range(nsup + 1):
                    if s < nsup:
                        scalar.wait_ge(sem_dve, t_dve_red(s))
                        nc.scalar.activation(
                            out=expw_t[:, CH0[s]:CH0[s + 1]],
                            in_=scores_t[:, CH0[s]:CH0[s + 1]],
                            func=ACTF.Exp,
                            accum_out=zc_t[:, s:s + 1],
                        ).then_inc(sem_act, 1)
                        if ka > 0:
                            nc.scalar.activation(
                                out=nexpw_t[:, CH0[s]:CH0[s + 1]],
                                in_=expw_t[:, CH0[s]:CH0[s + 1]],
                                func=ACTF.Copy, scale=-1.0,
                            )
                    if s >= 1 and ka_of[s - 1] > 0:
                        sm1 = s - 1
                        for j in range(ka_of[sm1]):
                            ga = ACT_IDX0[sm1] + j
                            if ga >= NAT:
                                po, co = ACT_LIST[ga - NAT]
                                scalar.wait_ge(sem_pe, t_pe_mm(po, co))
                            ca = CH0[sm1] + kd_of[sm1] + kg_of[sm1] + j
                            # u = (iota - bl)^2 ; A = Relu(expw*(1 - u))
                            nc.scalar.activation(
                                out=uat[:], in_=iota_t[:], func=ACTF.Square,
                                bias=bln_t[:, ca:ca + 1], scale=1.0,
                            )
                            nc.scalar.activation(
                                out=ata[ga % NAT][:], in_=uat[:], func=ACTF.Relu,
                                bias=expw_t[:, ca:ca + 1],
                                scale=nexpw_t[:, ca:ca + 1],
                            ).then_inc(sem_act, 1)
                    if s >= 1 and (s - 1) % spb == spb - 1:
                        b = (s - 1) // spb
                        sl = b * spb + spb - 1
                        scalar.wait_ge(sem_pe, t_pe_mm(sl, ch_of[sl] - 1))
                        nc.scalar.copy(
                            out=stage_t[:, b * D:(b + 1) * D], in_=pt[b % 2][:, 0:D]
                        ).then_inc(sem_cp, 1)

            @block.tensor
            def _(tensor):
                for sm1 in range(nsup):
                    b = sm1 // spb
                    j = sm1 % NXB
                    tensor.wait_ge(sem_x[j], 16 * (sm1 // NXB + 1))
                    if sm1 % spb == 0 and b >= 2:
                        tensor.wait_ge(sem_cp, b - 1)
                    for c in range(ch_of[sm1]):
                        if c < kd_of[sm1]:
                            tensor.wait_ge(sem_dve, t_dve_ts(sm1, c))
                            a = atd[(DVE_IDX0[sm1] + c) % NAT]
                        elif c < kd_of[sm1] + kg_of[sm1]:
                            i = c - kd_of[sm1]
                            tensor.wait_ge(sem_gp, t_gp_ts(sm1, i))
                            a = atg[(GP_IDX0[sm1] + i) % NAT]
                        else:
                            jx = c - kd_of[sm1] - kg_of[sm1]
                            tensor.wait_ge(sem_act, t_act_oh(sm1, jx))
                            a = ata[(ACT_IDX0[sm1] + jx) % NAT]
                        nc.tensor.matmul(
                            pt[b % 2][:, 0:D],
                            lhsT=a[:],
                            rhs=xt[j][:, c * D:(c + 1) * D],
                            start=(sm1 % spb == 0 and c == 0),
                            stop=(sm1 % spb == spb - 1 and c == ch_of[sm1] - 1),
                        ).then_inc(sem_pe, 1)

    return nc


def _pool(x, batch, W, num_graphs, n_cores=N_CORES, kd=None, ka=None):
    n = x.shape[0]
    segs_per_core = num_graphs // n_cores
    blocks = segs_per_core // P

    seg_starts = np.searchsorted(batch, np.arange(0, num_graphs + 1, P))
    blk_cnt = np.diff(seg_starts)
    blk_ch = max(1, int(np.ceil(blk_cnt.max() / P)))    # chunks per block
    n_b = blk_ch * P
    nch = blocks * blk_ch
    L = blocks * n_b
    sup_shape = []
    r = blk_ch
    while r > 0:
        t = min(SUP_CH, r)
        sup_shape.append(t)
        r -= t
    spb = len(sup_shape)
    nsup = blocks * spb
    if kd is None:
        kd = int(os.environ.get("KD", "7"))
    if ka is None:
        ka = int(os.environ.get("KA", "6"))

    x_bf = np.ascontiguousarray(x).astype(ml_dtypes.bfloat16)
    bloc_all = (batch % P).astype(np.float32)

    wrep = np.tile(np.asarray(W, np.float32).reshape(1, D), (P, SUP_CH)).astype(
        ml_dtypes.bfloat16)
    iota = np.broadcast_to(np.arange(P, dtype=np.float32), (P, P)).astype(
        ml_dtypes.bfloat16)

    in_maps, pad_counts = [], []
    for core in range(n_cores):
        xflat = np.zeros((L, D), ml_dtypes.bfloat16)
        blflat = np.full((L,), 999.0, np.float32)
        for bi in range(blocks):
            gb = core * blocks + bi
            s0, s1 = seg_starts[gb], seg_starts[gb + 1]
            cnt = s1 - s0
            xflat[bi * n_b: bi * n_b + cnt] = x_bf[s0:s1]
            blflat[bi * n_b: bi * n_b + cnt] = bloc_all[s0:s1]
        slabs = []
        off = 0
        for s in range(nsup):
            ch = sup_shape[s % spb]
            slabs.append(np.ascontiguousarray(
                xflat[off:off + ch * P].reshape(ch, P, D).transpose(1, 0, 2)
            ).reshape(-1))
            off += ch * P
        xp = np.concatenate(slabs)
        bl = np.ascontiguousarray(blflat.reshape(nch, P).T)
        pad_counts.append(L - int(blk_cnt[core * blocks:(core + 1) * blocks].sum()))
        in_maps.append({"xp": xp, "bl": bl, "bln": -bl, "wrep": wrep,
                        "iota": iota})

    key = (blocks, blk_ch, kd, ka)
    if key not in _prog_cache:
        _prog_cache[key] = _build(*key)
    nc = _prog_cache[key]

    res = run_bass_kernel_spmd(nc, in_maps, list(range(n_cores))).results

    z_total = 0.0
    parts = []
    for core in range(n_cores):
        z_total += float(res[core]["zout"].astype(np.float64).sum()) - pad_counts[core]
        o = res[core]["outp"].astype(np.float32)
        parts.append(o.reshape(P, blocks, D).transpose(1, 0, 2)
                     .reshape(segs_per_core, D))
    out = np.concatenate(parts, axis=0)
    return (out / np.float32(z_total)).astype(np.float32)


def kernel(x, batch, W, b):
    x = np.asarray(x, np.float32)
    batch = np.asarray(batch)
    W = np.asarray(W, np.float32)
    return _pool(x, batch, W, num_graphs=16384)


if __name__ == "__main__":
    rng = np.random.default_rng(0)
    G = 1024
    n = 16000
    x = rng.standard_normal((n, D), dtype=np.float32)
    batch = np.sort(rng.integers(0, G, n)).astype(np.int64)
    W = (rng.standard_normal((D, 1), dtype=np.float32) / np.sqrt(D)).astype(np.float32)
    b = np.zeros((1,), np.float32)

    got = _pool(x, batch, W, num_graphs=G)

    s = (x @ W).ravel()
    a = np.exp(s - s.max()); a /= a.sum()
    want = np.zeros((G, D), np.float64)
    np.add.at(want, batch, x * a[:, None])
    want = want.astype(np.float32)
    num = np.abs(got - want).max()
    print("abs err:", num, "rel err:", num / np.abs(want).max())



# revision 17
# speedup vs baseline: 1.7739x; 1.7739x over previous
"""AttentionPooling (global-softmax segment-sum) Trainium2 Bass kernel.

  scores = x @ W + b ; attn = softmax(scores, axis=0) ; out = segment_sum(x*attn, batch, G)

Design (8 cores, SPMD, raw Bass; softmax is shift-invariant so b drops out and
the fixed shift is 0; device computes unnormalized pooled sums + Z partials,
host divides at the end):

 * Segments are sorted by size (desc) and snake-dealt to the 8 cores, so every
   core sees a near-identical segment-size profile (cumulative node drift
   between cores < 1 chunk).  That allows ONE shared SPMD program in which
   chunk c of every core covers segments inside a shared window
   [W0(c), W0(c)+K) with small K (~4): the segment-scatter matrix per chunk is
   only [128, K] instead of a full [128, 128] one-hot.
 * x ships TRANSPOSED per 128-node chunk: xT_c [d=128 part, n=128 free] bf16,
   packed in 32-chunk DMA slabs (8 KB/partition lines -> full DMA efficiency).
 * PE per chunk (matmul operands in SBUF):
     scores:   mm(lhsT=xT_c, rhs=W[d,1])   -> psum col  [n,1] f32   (~2 ns)
     untrans:  PE transpose(xT_c)          -> psum x_c [n,d] BF16   (~53 ns)
     pooled:   mm(lhsT=x_c(sbuf), rhs=M_c[n,K]) += psum out[d, segcols]
               (start=False, banks double-buffered)                 (~2 ns)
   The pooled output lands TRANSPOSED [d, seg]; the host untransposes.
 * bf16 psum transposes pack 8 chunks per bank, so the psum->sbuf copies are
   [128, 1024] bf16 ops (2x mode on DVE) split across DVE and ACT.
 * ACT: Exp on 32-wide score strips (psum f32 -> sbuf expw f32).
 * DVE/GP: masks M_c = (iota_K == bl_c) * expw_c via one tensor_scalar
   [128,K] bf16 per chunk (~61/99 ns).  Z = one tensor_reduce over expw
   [128, nch] at the very end.

PSUM hazard rule (found the hard way; the device hangs otherwise): a bank PE
is writing must not be concurrently accessed by ACT/DVE.  Hence: scores
alternate between 2 banks per 32-chunk strip and PE re-enters a parity only
after that parity's previous Exp finished; transposed x rotates 4 banks
(copies read banks PE is not writing); the out accumulators are 2 banks
double-buffered over the (chunk-sequential) 512-segment ranges with
flush+memset strictly between PE uses.

TimelineSim (the graded cost model) is DMA-bound: the 65 MB/core bf16 x
stream at the modeled 360 GB/s is ~184 us.
"""

import hashlib
import os
import numpy as np
import ml_dtypes

import concourse.bass as bass
import concourse.mybir as mybir
from concourse.bass_utils import run_bass_kernel_spmd

BF16 = mybir.dt.bfloat16
F32 = mybir.dt.float32
ALU = mybir.AluOpType
ACTF = mybir.ActivationFunctionType

N_CORES = 8
P = 128
D = 128
SUP_CH = 32          # chunks per DMA super-slab
GRP = 8              # chunks per transpose-psum bank / copy op
NXB = 10             # xT slab ring depth
NT = 4               # transpose psum bank rotation
NXS = 12             # copied-back x_c sbuf slots (GRP-chunk groups)
NM = 128             # mask sbuf slots
LAG_G = 9            # pooled mms lag transposes by this many GRP-groups
EXPW = 32            # chunks per Exp strip
MLAG = 16            # exp/masks lag copies by this many chunks
KCAP = 16            # pass-1 span cap

_prog_cache = {}


def _build(nch, K, n_banks, bank_of, jb_of, sup_sizes, mask_dve, copy_eng):
    """Shared SPMD program.  bank_of/jb_of: per-chunk out range and column
    base.  sup_sizes: chunks per DMA super.  mask_dve[c]: mask built on DVE
    (else GPSIMD).  copy_eng[g]: 0=DVE 1=ACT for GRP-chunk psum->sbuf copies."""
    nsup = len(sup_sizes)
    CH0 = [0]
    for t in sup_sizes:
        CH0.append(CH0[-1] + t)
    assert CH0[-1] == nch and nch % GRP == 0
    ngrp = nch // GRP
    n_exp = (nch + EXPW - 1) // EXPW
    sup_of = []
    for s in range(nsup):
        sup_of += [s] * sup_sizes[s]

    # cumulative ticks
    mskd_tick = np.cumsum(mask_dve).tolist()
    mskg_tick = np.cumsum([not m for m in mask_dve]).tolist()
    cp_tick = [0] * ngrp
    cnt = [0, 0]
    for g in range(ngrp):
        cnt[copy_eng[g]] += 1
        cp_tick[g] = cnt[copy_eng[g]]

    G0 = [0]
    for c in range(nch):
        if bank_of[c] != len(G0) - 1:
            G0.append(c)
    G0 += [nch] * (n_banks + 1 - len(G0))

    nc = bass.Bass()
    xp_h = nc.declare_dram_parameter("xp", [nch * P * D], BF16, isOutput=False)
    bl_h = nc.declare_dram_parameter("bl", [P, nch], F32, isOutput=False)
    wcol_h = nc.declare_dram_parameter("wcol", [P, 1], BF16, isOutput=False)
    ident_h = nc.declare_dram_parameter("ident", [P, P], BF16, isOutput=False)
    iota_h = nc.declare_dram_parameter("iota", [P, K], BF16, isOutput=False)
    out_h = nc.declare_dram_parameter("outp", [P, n_banks * 512], F32, isOutput=True)
    z_h = nc.declare_dram_parameter("zout", [P, 1], F32, isOutput=True)

    import contextlib
    with contextlib.ExitStack() as ctx:
        sem_x = [ctx.enter_context(nc.semaphore(f"sem_x{j}")) for j in range(NXB)]
        sem_cst = ctx.enter_context(nc.semaphore("sem_cst"))
        sem_sc = ctx.enter_context(nc.semaphore("sem_sc"))
        sem_tr = ctx.enter_context(nc.semaphore("sem_tr"))
        sem_ex = ctx.enter_context(nc.semaphore("sem_ex"))
        sem_md = ctx.enter_context(nc.semaphore("sem_md"))
        sem_mg = ctx.enter_context(nc.semaphore("sem_mg"))
        sem_pl = ctx.enter_context(nc.semaphore("sem_pl"))
        sem_cp = [ctx.enter_context(nc.semaphore(f"sem_cp{e}")) for e in range(2)]
        sem_ini = ctx.enter_context(nc.semaphore("sem_ini"))
        sem_zr = ctx.enter_context(nc.semaphore("sem_zr"))
        sem_fl = ctx.enter_context(nc.semaphore("sem_fl"))
        sem_out = ctx.enter_context(nc.semaphore("sem_out"))

        xt = [ctx.enter_context(nc.sbuf_tensor(f"xt{j}", [P, SUP_CH * D], BF16))
              for j in range(NXB)]
        wcol_t = ctx.enter_context(nc.sbuf_tensor("wcol_t", [P, 1], BF16))
        ident_t = ctx.enter_context(nc.sbuf_tensor("ident_t", [P, P], BF16))
        iota_t = ctx.enter_context(nc.sbuf_tensor("iota_t", [P, K], BF16))
        bl_t = ctx.enter_context(nc.sbuf_tensor("bl_t", [P, nch], F32))
        expw_t = ctx.enter_context(nc.sbuf_tensor("expw_t", [P, nch], F32))
        xsb = [ctx.enter_context(nc.sbuf_tensor(f"xsb{j}", [P, GRP * D], BF16))
               for j in range(NXS)]
        msk = [ctx.enter_context(nc.sbuf_tensor(f"msk{j}", [P, K], BF16))
               for j in range(NM)]
        stage_t = ctx.enter_context(nc.sbuf_tensor("stage_t", [P, n_banks * 512], F32))
        zsum_t = ctx.enter_context(nc.sbuf_tensor("zsum_t", [P, 1], F32))

        # PSUM hazard rule: a bank PE is writing must never be concurrently
        # accessed by ACT/DVE (the device hangs).  Scores: 2 banks alternated
        # per strip; transposes: 4 bf16 banks of GRP chunks; out: 2 banks
        # double-buffered over the sequential 512-seg ranges.
        sp2 = [ctx.enter_context(nc.psum_tensor(f"sp{i}", [P, 512], F32))
               for i in range(2)]
        tp = [ctx.enter_context(nc.psum_tensor(f"tp{j}", [P, GRP * D], BF16))
              for j in range(NT)]
        outp2 = [ctx.enter_context(nc.psum_tensor(f"op{b}", [P, 512], F32))
                 for b in range(2)]

        N_CST = 4  # preamble DMAs

        def sploc(c):
            e = c // EXPW
            return sp2[e % 2], ((e // 2) * EXPW) % 512 + (c % EXPW)

        def pooled_group(tensor, go, tail):
            c0 = GRP * go
            if c0 % EXPW == 0:
                ce = min(c0 + EXPW, nch) - 1
                tensor.wait_ge(sem_md, mskd_tick[ce])
                tensor.wait_ge(sem_mg, mskg_tick[ce])
            if tail and go >= ngrp - NT:
                tensor.wait_ge(sem_cp[copy_eng[go]], cp_tick[go])
            for cc in range(c0, c0 + GRP):
                r = bank_of[cc]
                if r >= 2 and cc == G0[r]:
                    tensor.wait_ge(sem_ini, r + 1)   # memset of reused bank
                nc.tensor.matmul(
                    outp2[r % 2][:, jb_of[cc]:jb_of[cc] + K],
                    lhsT=xsb[go % NXS][:, (cc % GRP) * D:(cc % GRP + 1) * D],
                    rhs=msk[cc % NM][:],
                    start=False, stop=True, skip_group_check=True,
                ).then_inc(sem_pl, 1)

        with nc.Block() as block:

            @block.sync
            def _(sync):
                sync.dma_start(out=wcol_t[:], in_=wcol_h[:]).then_inc(sem_cst, 16)
                sync.dma_start(out=ident_t[:], in_=ident_h[:]).then_inc(sem_cst, 16)
                sync.dma_start(out=iota_t[:], in_=iota_h[:]).then_inc(sem_cst, 16)
                sync.dma_start(out=bl_t[:], in_=bl_h[:]).then_inc(sem_cst, 16)
                for s in range(nsup):
                    j = s % NXB
                    ch = sup_sizes[s]
                    if s >= NXB:
                        sync.wait_ge(sem_tr, CH0[s - NXB + 1])
                    sync.dma_start(
                        out=xt[j][:, 0:ch * D],
                        in_=xp_h[CH0[s] * P * D:CH0[s + 1] * P * D].rearrange(
                            "(d f) -> d f", d=P),
                    ).then_inc(sem_x[j], 16)
                sync.wait_ge(sem_zr, 1)
                sync.dma_start(out=z_h[:], in_=zsum_t[:]).then_inc(sem_out, 16)
                for b in range(n_banks):
                    sync.wait_ge(sem_fl, b + 1)
                    sync.dma_start(
                        out=out_h[:, b * 512:(b + 1) * 512],
                        in_=stage_t[:, b * 512:(b + 1) * 512],
                    ).then_inc(sem_out, 16)
                sync.wait_ge(sem_out, 16 * (n_banks + 1))

            @block.tensor
            def _(tensor):
                tensor.wait_ge(sem_cst, 16 * N_CST)
                tensor.wait_ge(sem_ini, 2)
                for c in range(nch):
                    s = sup_of[c]
                    ci = c - CH0[s]
                    if ci == 0:
                        tensor.wait_ge(sem_x[s % NXB], 16 * (s // NXB + 1))
                    if c % EXPW == 0 and c // EXPW >= 2:
                        # reuse of this parity's score bank: prior strip's Exp
                        tensor.wait_ge(sem_ex, c // EXPW - 1)
                    xsl = xt[s % NXB][:, ci * D:(ci + 1) * D]
                    bnk, col = sploc(c)
                    nc.tensor.matmul(
                        bnk[:, col:col + 1],
                        lhsT=xsl, rhs=wcol_t[:],
                        start=True, stop=True, skip_group_check=True,
                    ).then_inc(sem_sc, 1)
                    g = c // GRP
                    if c % GRP == 0 and g >= NT:
                        go2 = g - NT
                        tensor.wait_ge(sem_cp[copy_eng[go2]], cp_tick[go2])
                    nc.tensor.transpose(
                        tp[g % NT][:, (c % GRP) * D:(c % GRP + 1) * D],
                        xsl, ident_t[:],
                    ).then_inc(sem_tr, 1)
                    if c % GRP == GRP - 1 and g >= LAG_G:
                        pooled_group(tensor, g - LAG_G, False)
                for go in range(max(0, ngrp - LAG_G), ngrp):
                    pooled_group(tensor, go, True)

            # Copies run at position p; exp/masks trail at p-MLAG so neither
            # ACT nor DVE blocks on exp before emitting a copy PE waits on.

            @block.scalar
            def _(scalar):
                nfl = 0
                for p in range(0, nch + MLAG, GRP):
                    g = p // GRP
                    if g < ngrp and copy_eng[g] == 1:
                        scalar.wait_ge(sem_tr, GRP * g + GRP)
                        if g >= NXS:
                            scalar.wait_ge(sem_pl, GRP * (g - NXS) + GRP)
                        nc.scalar.copy(
                            out=xsb[g % NXS][:], in_=tp[g % NT][:],
                        ).then_inc(sem_cp[1], 1)
                    cm = p - MLAG
                    if cm >= 0 and cm % EXPW == 0:
                        e = cm // EXPW
                        c0, c1 = EXPW * e, min(EXPW * e + EXPW, nch)
                        scalar.wait_ge(sem_sc, c1)
                        bnk, col = sploc(c0)
                        nc.scalar.activation(
                            out=expw_t[:, c0:c1],
                            in_=bnk[:, col:col + (c1 - c0)],
                            func=ACTF.Exp,
                        ).then_inc(sem_ex, 1)
                    while nfl < n_banks and G0[nfl + 1] + 80 <= p:
                        scalar.wait_ge(sem_pl, G0[nfl + 1])
                        nc.scalar.copy(
                            out=stage_t[:, nfl * 512:(nfl + 1) * 512],
                            in_=outp2[nfl % 2][:],
                        ).then_inc(sem_fl, 1)
                        nfl += 1
                while nfl < n_banks:
                    scalar.wait_ge(sem_pl, G0[nfl + 1])
                    nc.scalar.copy(
                        out=stage_t[:, nfl * 512:(nfl + 1) * 512],
                        in_=outp2[nfl % 2][:],
                    ).then_inc(sem_fl, 1)
                    nfl += 1

            @block.vector
            def _(vector):
                for b in range(2):
                    nc.vector.memset(outp2[b][:], 0.0).then_inc(sem_ini, 1)
                vector.wait_ge(sem_cst, 16 * N_CST)
                nms = 2
                for p in range(nch + MLAG):
                    g = p // GRP
                    if p < nch and p % GRP == GRP - 1 and copy_eng[g] == 0:
                        vector.wait_ge(sem_tr, GRP * g + GRP)
                        if g >= NXS:
                            vector.wait_ge(sem_pl, GRP * (g - NXS) + GRP)
                        nc.vector.tensor_copy(
                            out=xsb[g % NXS][:], in_=tp[g % NT][:],
                        ).then_inc(sem_cp[0], 1)
                    while nms < n_banks and G0[nms - 1] + 96 <= p:
                        vector.wait_ge(sem_fl, nms - 1)
                        nc.vector.memset(outp2[nms % 2][:], 0.0).then_inc(sem_ini, 1)
                        nms += 1
                    cm = p - MLAG
                    if cm < 0:
                        continue
                    if cm % EXPW == 0:
                        vector.wait_ge(sem_ex, cm // EXPW + 1)
                        if cm >= NM:
                            vector.wait_ge(sem_pl, cm - NM + 1)
                    if mask_dve[cm]:
                        nc.vector.tensor_scalar(
                            msk[cm % NM][:], iota_t[:],
                            bl_t[:, cm:cm + 1], expw_t[:, cm:cm + 1],
                            ALU.is_equal, ALU.mult,
                        ).then_inc(sem_md, 1)
                # Z = sum over all chunks of expw (pads contribute e^0=1 each;
                # host subtracts the pad count)
                vector.wait_ge(sem_ex, n_exp)
                nc.vector.tensor_reduce(
                    out=zsum_t[:], in_=expw_t[:],
                    axis=mybir.AxisListType.X, op=ALU.add,
                ).then_inc(sem_zr, 1)

            @block.gpsimd
            def _(gpsimd):
                # GPSIMD cannot access PSUM: masks only
                gpsimd.wait_ge(sem_cst, 16 * N_CST)
                for cm in range(nch):
                    if cm % EXPW == 0:
                        gpsimd.wait_ge(sem_ex, cm // EXPW + 1)
                        if cm >= NM:
                            gpsimd.wait_ge(sem_pl, cm - NM + 1)
                    if not mask_dve[cm]:
                        nc.gpsimd.tensor_scalar(
                            msk[cm % NM][:], iota_t[:],
                            bl_t[:, cm:cm + 1], expw_t[:, cm:cm + 1],
                            ALU.is_equal, ALU.mult,
                        ).then_inc(sem_mg, 1)

    return nc


def _plan(counts_k, n_banks):
    """Pass-1 chunking for one core: counts_k[j] = node count of local seg j.
    Returns per-group chunk lists [(jf, [(j, off, take), ...]), ...]."""
    groups = []
    nsegs = len(counts_k)
    for gb in range(n_banks):
        glo, ghi = 512 * gb, min(512 * (gb + 1), nsegs)
        chunks = []
        cur_nodes, cur_jf, cur_n = [], None, 0
        for j in range(glo, ghi):
            cnt = int(counts_k[j])
            off = 0
            while cnt > 0:
                if cur_jf is not None and j - cur_jf + 1 > KCAP:
                    chunks.append((cur_jf, cur_nodes))
                    cur_nodes, cur_jf, cur_n = [], None, 0
                if cur_jf is None:
                    cur_jf = j
                take = min(cnt, P - cur_n)
                cur_nodes.append((j, off, take))
                cur_n += take
                off += take
                cnt -= take
                if cur_n == P:
                    chunks.append((cur_jf, cur_nodes))
                    cur_nodes, cur_jf, cur_n = [], None, 0
        if cur_n > 0:
            chunks.append((cur_jf, cur_nodes))
        groups.append(chunks)
    return groups


def _pool(x, batch, W, num_graphs, n_cores=N_CORES):
    segs_per_core = num_graphs // n_cores
    n_banks = (segs_per_core + 511) // 512

    counts = np.bincount(batch, minlength=num_graphs).astype(np.int64)
    order = np.argsort(-counts, kind="stable")      # global seg ids, size desc
    orig_starts = np.zeros(num_graphs + 1, np.int64)
    np.cumsum(counts, out=orig_starts[1:])

    # snake deal: sorted position p -> (core, local j)
    nloc = num_graphs // n_cores
    pos = np.arange(num_graphs).reshape(nloc, n_cores)
    core_of_pos = np.where((np.arange(nloc) % 2 == 0)[:, None],
                           np.arange(n_cores)[None, :],
                           np.arange(n_cores)[None, :][:, ::-1])
    local_ids = np.empty((n_cores, nloc), np.int64)
    for k in range(n_cores):
        local_ids[k] = order[pos[core_of_pos == k]]
    local_counts = counts[local_ids]                # [n_cores, nloc]

    plans = [_plan(local_counts[k], n_banks) for k in range(n_cores)]
    ngc = [max(len(plans[k][g]) for k in range(n_cores)) for g in range(n_banks)]
    total = sum(ngc)
    ngc[-1] += (-total) % GRP
    nch = sum(ngc)

    G0 = [0]
    for t in ngc:
        G0.append(G0[-1] + t)
    W0 = np.full(nch, np.iinfo(np.int64).max, np.int64)
    W1 = np.full(nch, -1, np.int64)
    for k in range(n_cores):
        for g in range(n_banks):
            for i, (jf, nodes) in enumerate(plans[k][g]):
                c = G0[g] + i
                W0[c] = min(W0[c], jf)
                W1[c] = max(W1[c], nodes[-1][0])
    bank_of = np.empty(nch, np.int64)
    for g in range(n_banks):
        bank_of[G0[g]:G0[g + 1]] = g
        empt = W1[G0[g]:G0[g + 1]] < 0            # all-core-empty pad chunks
        W0[G0[g]:G0[g + 1]][empt] = 512 * g
        W1[G0[g]:G0[g + 1]][empt] = 512 * g
    K = int(max(2, (W1 - W0).max() + 1))
    jb_of = np.minimum(W0 - 512 * bank_of, 512 - K).astype(np.int64)
    assert jb_of.min() >= 0

    sup_sizes = [SUP_CH] * (nch // SUP_CH)
    if nch % SUP_CH:
        sup_sizes.append(nch % SUP_CH)

    # engine splits (tunable): masks on DVE (frac MD) else GPSIMD;
    # psum->sbuf copies: CPAT cycled over GRP-chunk groups (0=DVE, 1=ACT)
    mfrac = float(os.environ.get("MD", "0.3125"))
    mask_dve = [(int(c * mfrac) != int((c + 1) * mfrac)) for c in range(nch)]
    ngrp = nch // GRP
    cpat = [int(v) for v in os.environ.get("CPAT", "0,1,0,0,1,0,0,1").split(",")]
    copy_eng = [cpat[g % len(cpat)] for g in range(ngrp)]

    # per-core tensors
    x_bf = np.ascontiguousarray(x).astype(ml_dtypes.bfloat16)
    in_maps, pad_counts = [], []
    for k in range(n_cores):
        xflat = np.zeros((nch * P, D), ml_dtypes.bfloat16)
        blflat = np.full((nch * P,), 999.0, np.float32)
        real = 0
        for g in range(n_banks):
            for i, (jf, nodes) in enumerate(plans[k][g]):
                c = G0[g] + i
                base = 512 * bank_of[c] + jb_of[c]
                p0 = c * P
                for (j, off, take) in nodes:
                    gid = local_ids[k][j]
                    s0 = orig_starts[gid] + off
                    xflat[p0:p0 + take] = x_bf[s0:s0 + take]
                    blflat[p0:p0 + take] = j - base
                    p0 += take
                    real += take
        pad_counts.append(nch * P - real)
        # slab per super: (c, n, d) -> (d, c, n)
        slabs = []
        o = 0
        for ch in sup_sizes:
            a = xflat[o * P:(o + ch) * P]
            slabs.append(np.ascontiguousarray(
                a.reshape(ch, P, D).transpose(2, 0, 1)).reshape(-1))
            o += ch
        xp = np.concatenate(slabs)
        bl = np.ascontiguousarray(blflat.reshape(nch, P).T).astype(np.float32)
        in_maps.append({
            "xp": xp, "bl": bl,
            "wcol": np.asarray(W, np.float32).reshape(P, 1).astype(ml_dtypes.bfloat16),
            "ident": np.eye(P, dtype=ml_dtypes.bfloat16),
            "iota": np.broadcast_to(
                np.arange(K).astype(ml_dtypes.bfloat16), (P, K)).copy(),
        })

    key = hashlib.sha1(
        np.concatenate([bank_of, jb_of, [nch, K, n_banks]]).tobytes()
        + bytes(mask_dve) + bytes(copy_eng) + bytes(str(sup_sizes), "ascii")
    ).hexdigest()
    if key not in _prog_cache:
        _prog_cache[key] = _build(nch, K, n_banks, bank_of.tolist(),
                                  jb_of.tolist(), sup_sizes, mask_dve, copy_eng)
    nc = _prog_cache[key]

    res = run_bass_kernel_spmd(nc, in_maps, list(range(n_cores))).results

    z_total = 0.0
    out = np.zeros((num_graphs, D), np.float32)
    for k in range(n_cores):
        z_total += float(res[k]["zout"].astype(np.float64).sum()) - pad_counts[k]
        o = res[k]["outp"].astype(np.float32)       # [D, n_banks*512]
        out[local_ids[k]] = o.T[:nloc]
    return (out / np.float32(z_total)).astype(np.float32)


def kernel(x, batch, W, b):
    x = np.asarray(x, np.float32)
    batch = np.asarray(batch).astype(np.int64)
    W = np.asarray(W, np.float32)
    return _pool(x, batch, W, num_graphs=16384)


if __name__ == "__main__":
    rng = np.random.default_rng(0)
    G = int(os.environ.get("TG", "1024"))
    n = int(os.environ.get("TN", "64000"))
    x = rng.standard_normal((n, D), dtype=np.float32)
    batch = np.sort(rng.integers(0, G, n)).astype(np.int64)
    W = (rng.standard_normal((D, 1), dtype=np.float32) / np.sqrt(D)).astype(np.float32)
    b = np.zeros((1,), np.float32)

    got = _pool(x, batch, W, num_graphs=G)

    s = (x @ W).ravel()
    a = np.exp(s - s.max()); a /= a.sum()
    want = np.zeros((G, D), np.float64)
    np.add.at(want, batch, x * a[:, None])
    want = want.astype(np.float32)
    num = np.abs(got - want).max()
    print("abs err:", num, "rel err:", num / np.abs(want).max())


# revision 20
# speedup vs baseline: 1.8250x; 1.0288x over previous
"""AttentionPooling (global-softmax segment-sum) Trainium2 Bass kernel.

  scores = x @ W + b ; attn = softmax(scores, axis=0) ; out = segment_sum(x*attn, batch, G)

Design (8 cores, SPMD, raw Bass; softmax is shift-invariant so b drops out and
the fixed shift is 0; device computes unnormalized pooled sums + Z partials,
host divides at the end):

 * Segments are sorted by size (desc) and snake-dealt to the 8 cores, so every
   core sees a near-identical segment-size profile (cumulative node drift
   between cores < 1 chunk).  That allows ONE shared SPMD program in which
   chunk c of every core covers segments inside a shared window
   [W0(c), W0(c)+K) with small K (~4): the segment-scatter matrix per chunk is
   only [128, K] instead of a full [128, 128] one-hot.
 * x ships TRANSPOSED per 128-node chunk: xT_c [d=128 part, n=128 free] bf16,
   packed in 32-chunk DMA slabs (8 KB/partition lines -> full DMA efficiency).
 * PE per chunk (matmul operands in SBUF):
     scores:   mm(lhsT=xT_c, rhs=W[d,1])   -> psum col  [n,1] f32   (~2 ns)
     untrans:  PE transpose(xT_c)          -> psum x_c [n,d] BF16   (~53 ns)
     pooled:   mm(lhsT=x_c(sbuf), rhs=M_c[n,K]) += psum out[d, segcols]
               (start=False, banks double-buffered)                 (~2 ns)
   The pooled output lands TRANSPOSED [d, seg]; the host untransposes.
 * bf16 psum transposes pack 8 chunks per bank, so the psum->sbuf copies are
   [128, 1024] bf16 ops (2x mode on DVE) split across DVE and ACT.
 * ACT: Exp on 32-wide score strips (psum f32 -> sbuf expw f32).
 * DVE/GP: masks M_c = (iota_K == bl_c) * expw_c via one tensor_scalar
   [128,K] bf16 per chunk (~61/99 ns).  Z = one tensor_reduce over expw
   [128, nch] at the very end.

PSUM hazard rule (found the hard way; the device hangs otherwise): a bank PE
is writing must not be concurrently accessed by ACT/DVE.  Hence: scores
alternate between 2 banks per 32-chunk strip and PE re-enters a parity only
after that parity's previous Exp finished; transposed x rotates 4 banks
(copies read banks PE is not writing); the out accumulators are 2 banks
double-buffered over the (chunk-sequential) 512-segment ranges with
flush+memset strictly between PE uses.

TimelineSim (the graded cost model) is DMA-bound: the 65 MB/core bf16 x
stream at the modeled 360 GB/s is ~184 us.
"""

import hashlib
import os
import numpy as np
import ml_dtypes

import concourse.bass as bass
import concourse.mybir as mybir
from concourse.bass_utils import run_bass_kernel_spmd

BF16 = mybir.dt.bfloat16
F32 = mybir.dt.float32
ALU = mybir.AluOpType
ACTF = mybir.ActivationFunctionType

N_CORES = 8
P = 128
D = 128
SUP_CH = 32          # chunks per DMA super-slab
GRP = 8              # chunks per transpose-psum bank / copy op
NXB = 10             # xT slab ring depth
NT = 4               # transpose psum bank rotation
NXS = 12             # copied-back x_c sbuf slots (GRP-chunk groups)
NM = 128             # mask sbuf slots
LAG_G = 9            # pooled mms lag transposes by this many GRP-groups
EXPW = 32            # chunks per Exp strip
MLAG = 16            # exp/masks lag copies by this many chunks
KCAP = 16            # pass-1 span cap

_prog_cache = {}


def _build(nch, K, n_banks, bank_of, jb_of, sup_sizes, mask_dve, copy_eng):
    """Shared SPMD program.  bank_of/jb_of: per-chunk out range and column
    base.  sup_sizes: chunks per DMA super.  mask_dve[c]: mask built on DVE
    (else GPSIMD).  copy_eng[g]: 0=DVE 1=ACT for GRP-chunk psum->sbuf copies."""
    nsup = len(sup_sizes)
    CH0 = [0]
    for t in sup_sizes:
        CH0.append(CH0[-1] + t)
    assert CH0[-1] == nch and nch % GRP == 0
    ngrp = nch // GRP
    n_exp = (nch + EXPW - 1) // EXPW
    sup_of = []
    for s in range(nsup):
        sup_of += [s] * sup_sizes[s]

    # cumulative ticks
    mskd_tick = np.cumsum(mask_dve).tolist()
    mskg_tick = np.cumsum([not m for m in mask_dve]).tolist()
    cp_tick = [0] * ngrp
    cnt = [0, 0]
    for g in range(ngrp):
        cnt[copy_eng[g]] += 1
        cp_tick[g] = cnt[copy_eng[g]]

    G0 = [0]
    for c in range(nch):
        if bank_of[c] != len(G0) - 1:
            G0.append(c)
    G0 += [nch] * (n_banks + 1 - len(G0))

    nc = bass.Bass()
    xp_h = nc.declare_dram_parameter("xp", [nch * P * D], BF16, isOutput=False)
    bl_h = nc.declare_dram_parameter("bl", [P, nch], F32, isOutput=False)
    wcol_h = nc.declare_dram_parameter("wcol", [P, 1], BF16, isOutput=False)
    ident_h = nc.declare_dram_parameter("ident", [P, P], BF16, isOutput=False)
    iota_h = nc.declare_dram_parameter("iota", [P, K], BF16, isOutput=False)
    out_h = nc.declare_dram_parameter("outp", [P, n_banks * 512], F32, isOutput=True)
    z_h = nc.declare_dram_parameter("zout", [P, 1], F32, isOutput=True)

    import contextlib
    with contextlib.ExitStack() as ctx:
        sem_x = [ctx.enter_context(nc.semaphore(f"sem_x{j}")) for j in range(NXB)]
        sem_cst = ctx.enter_context(nc.semaphore("sem_cst"))
        sem_sc = ctx.enter_context(nc.semaphore("sem_sc"))
        sem_tr = ctx.enter_context(nc.semaphore("sem_tr"))
        sem_ex = ctx.enter_context(nc.semaphore("sem_ex"))
        sem_md = ctx.enter_context(nc.semaphore("sem_md"))
        sem_mg = ctx.enter_context(nc.semaphore("sem_mg"))
        sem_pl = ctx.enter_context(nc.semaphore("sem_pl"))
        sem_cp = [ctx.enter_context(nc.semaphore(f"sem_cp{e}")) for e in range(2)]
        sem_ini = ctx.enter_context(nc.semaphore("sem_ini"))
        sem_zr = ctx.enter_context(nc.semaphore("sem_zr"))
        sem_fl = ctx.enter_context(nc.semaphore("sem_fl"))
        sem_out = ctx.enter_context(nc.semaphore("sem_out"))

        xt = [ctx.enter_context(nc.sbuf_tensor(f"xt{j}", [P, SUP_CH * D], BF16))
              for j in range(NXB)]
        wcol_t = ctx.enter_context(nc.sbuf_tensor("wcol_t", [P, 1], BF16))
        ident_t = ctx.enter_context(nc.sbuf_tensor("ident_t", [P, P], BF16))
        iota_t = ctx.enter_context(nc.sbuf_tensor("iota_t", [P, K], BF16))
        bl_t = ctx.enter_context(nc.sbuf_tensor("bl_t", [P, nch], F32))
        expw_t = ctx.enter_context(nc.sbuf_tensor("expw_t", [P, nch], F32))
        xsb = [ctx.enter_context(nc.sbuf_tensor(f"xsb{j}", [P, GRP * D], BF16))
               for j in range(NXS)]
        msk = [ctx.enter_context(nc.sbuf_tensor(f"msk{j}", [P, K], BF16))
               for j in range(NM)]
        stage_t = ctx.enter_context(nc.sbuf_tensor("stage_t", [P, n_banks * 512], F32))
        zsum_t = ctx.enter_context(nc.sbuf_tensor("zsum_t", [P, 1], F32))

        # PSUM hazard rule: a bank PE is writing must never be concurrently
        # accessed by ACT/DVE (the device hangs).  Scores: 2 banks alternated
        # per strip; transposes: 4 bf16 banks of GRP chunks; out: 2 banks
        # double-buffered over the sequential 512-seg ranges.
        sp2 = [ctx.enter_context(nc.psum_tensor(f"sp{i}", [P, 512], F32))
               for i in range(2)]
        tp = [ctx.enter_context(nc.psum_tensor(f"tp{j}", [P, GRP * D], BF16))
              for j in range(NT)]
        outp2 = [ctx.enter_context(nc.psum_tensor(f"op{b}", [P, 512], F32))
                 for b in range(2)]

        N_CST = 4  # preamble DMAs

        def sploc(c):
            e = c // EXPW
            return sp2[e % 2], ((e // 2) * EXPW) % 512 + (c % EXPW)

        def pooled_group(tensor, go, tail):
            c0 = GRP * go
            if c0 % EXPW == 0:
                ce = min(c0 + EXPW, nch) - 1
                tensor.wait_ge(sem_md, mskd_tick[ce])
                tensor.wait_ge(sem_mg, mskg_tick[ce])
            if tail and go >= ngrp - NT:
                tensor.wait_ge(sem_cp[copy_eng[go]], cp_tick[go])
            for cc in range(c0, c0 + GRP):
                r = bank_of[cc]
                if r >= 2 and cc == G0[r]:
                    tensor.wait_ge(sem_ini, r + 1)   # memset of reused bank
                nc.tensor.matmul(
                    outp2[r % 2][:, jb_of[cc]:jb_of[cc] + K],
                    lhsT=xsb[go % NXS][:, (cc % GRP) * D:(cc % GRP + 1) * D],
                    rhs=msk[cc % NM][:],
                    start=False, stop=True, skip_group_check=True,
                ).then_inc(sem_pl, 1)

        with nc.Block() as block:

            @block.sync
            def _(sync):
                sync.dma_start(out=wcol_t[:], in_=wcol_h[:]).then_inc(sem_cst, 16)
                sync.dma_start(out=ident_t[:], in_=ident_h[:]).then_inc(sem_cst, 16)
                sync.dma_start(out=iota_t[:], in_=iota_h[:]).then_inc(sem_cst, 16)
                sync.dma_start(out=bl_t[:], in_=bl_h[:]).then_inc(sem_cst, 16)
                for s in range(nsup):
                    j = s % NXB
                    ch = sup_sizes[s]
                    if s >= NXB:
                        sync.wait_ge(sem_tr, CH0[s - NXB + 1])
                    sync.dma_start(
                        out=xt[j][:, 0:ch * D],
                        in_=xp_h[CH0[s] * P * D:CH0[s + 1] * P * D].rearrange(
                            "(d f) -> d f", d=P),
                    ).then_inc(sem_x[j], 16)
                sync.wait_ge(sem_zr, 1)
                sync.dma_start(out=z_h[:], in_=zsum_t[:]).then_inc(sem_out, 16)
                for b in range(n_banks):
                    sync.wait_ge(sem_fl, b + 1)
                    sync.dma_start(
                        out=out_h[:, b * 512:(b + 1) * 512],
                        in_=stage_t[:, b * 512:(b + 1) * 512],
                    ).then_inc(sem_out, 16)
                sync.wait_ge(sem_out, 16 * (n_banks + 1))

            @block.tensor
            def _(tensor):
                tensor.wait_ge(sem_cst, 16 * N_CST)
                tensor.wait_ge(sem_ini, 2)
                for c in range(nch):
                    s = sup_of[c]
                    ci = c - CH0[s]
                    if ci == 0:
                        tensor.wait_ge(sem_x[s % NXB], 16 * (s // NXB + 1))
                    if c % EXPW == 0 and c // EXPW >= 2:
                        # reuse of this parity's score bank: prior strip's Exp
                        tensor.wait_ge(sem_ex, c // EXPW - 1)
                    xsl = xt[s % NXB][:, ci * D:(ci + 1) * D]
                    bnk, col = sploc(c)
                    nc.tensor.matmul(
                        bnk[:, col:col + 1],
                        lhsT=xsl, rhs=wcol_t[:],
                        start=True, stop=True, skip_group_check=True,
                    ).then_inc(sem_sc, 1)
                    g = c // GRP
                    if c % GRP == 0 and g >= NT:
                        go2 = g - NT
                        tensor.wait_ge(sem_cp[copy_eng[go2]], cp_tick[go2])
                    nc.tensor.transpose(
                        tp[g % NT][:, (c % GRP) * D:(c % GRP + 1) * D],
                        xsl, ident_t[:],
                    ).then_inc(sem_tr, 1)
                    if c % GRP == GRP - 1 and g >= LAG_G:
                        pooled_group(tensor, g - LAG_G, False)
                for go in range(max(0, ngrp - LAG_G), ngrp):
                    pooled_group(tensor, go, True)

            # Copies run at position p; exp/masks trail at p-MLAG so neither
            # ACT nor DVE blocks on exp before emitting a copy PE waits on.

            @block.scalar
            def _(scalar):
                nfl = 0
                for p in range(0, nch + MLAG, GRP):
                    g = p // GRP
                    if g < ngrp and copy_eng[g] == 1:
                        scalar.wait_ge(sem_tr, GRP * g + GRP)
                        if g >= NXS:
                            scalar.wait_ge(sem_pl, GRP * (g - NXS) + GRP)
                        nc.scalar.copy(
                            out=xsb[g % NXS][:], in_=tp[g % NT][:],
                        ).then_inc(sem_cp[1], 1)
                    cm = p - MLAG
                    if cm >= 0 and cm % EXPW == 0:
                        e = cm // EXPW
                        c0, c1 = EXPW * e, min(EXPW * e + EXPW, nch)
                        scalar.wait_ge(sem_sc, c1)
                        bnk, col = sploc(c0)
                        nc.scalar.activation(
                            out=expw_t[:, c0:c1],
                            in_=bnk[:, col:col + (c1 - c0)],
                            func=ACTF.Exp,
                        ).then_inc(sem_ex, 1)
                    while nfl < n_banks and G0[nfl + 1] + 80 <= p:
                        scalar.wait_ge(sem_pl, G0[nfl + 1])
                        nc.scalar.copy(
                            out=stage_t[:, nfl * 512:(nfl + 1) * 512],
                            in_=outp2[nfl % 2][:],
                        ).then_inc(sem_fl, 1)
                        nfl += 1
                while nfl < n_banks:
                    scalar.wait_ge(sem_pl, G0[nfl + 1])
                    nc.scalar.copy(
                        out=stage_t[:, nfl * 512:(nfl + 1) * 512],
                        in_=outp2[nfl % 2][:],
                    ).then_inc(sem_fl, 1)
                    nfl += 1

            @block.vector
            def _(vector):
                for b in range(2):
                    nc.vector.memset(outp2[b][:], 0.0).then_inc(sem_ini, 1)
                vector.wait_ge(sem_cst, 16 * N_CST)
                nms = 2
                for p in range(nch + MLAG):
                    g = p // GRP
                    if p < nch and p % GRP == GRP - 1 and copy_eng[g] == 0:
                        vector.wait_ge(sem_tr, GRP * g + GRP)
                        if g >= NXS:
                            vector.wait_ge(sem_pl, GRP * (g - NXS) + GRP)
                        nc.vector.tensor_copy(
                            out=xsb[g % NXS][:], in_=tp[g % NT][:],
                        ).then_inc(sem_cp[0], 1)
                    while nms < n_banks and G0[nms - 1] + 96 <= p:
                        vector.wait_ge(sem_fl, nms - 1)
                        nc.vector.memset(outp2[nms % 2][:], 0.0).then_inc(sem_ini, 1)
                        nms += 1
                    cm = p - MLAG
                    if cm < 0:
                        continue
                    if cm % EXPW == 0:
                        vector.wait_ge(sem_ex, cm // EXPW + 1)
                        if cm >= NM:
                            vector.wait_ge(sem_pl, cm - NM + 1)
                    if mask_dve[cm]:
                        nc.vector.tensor_scalar(
                            msk[cm % NM][:], iota_t[:],
                            bl_t[:, cm:cm + 1], expw_t[:, cm:cm + 1],
                            ALU.is_equal, ALU.mult,
                        ).then_inc(sem_md, 1)
                # Z = sum over all chunks of expw (pads contribute e^0=1
                # each; host subtracts the pad count)
                vector.wait_ge(sem_ex, n_exp)
                nc.vector.tensor_reduce(
                    out=zsum_t[:], in_=expw_t[:],
                    axis=mybir.AxisListType.X, op=ALU.add,
                ).then_inc(sem_zr, 1)

            @block.gpsimd
            def _(gpsimd):
                # GPSIMD cannot access PSUM: masks only
                gpsimd.wait_ge(sem_cst, 16 * N_CST)
                for cm in range(nch):
                    if cm % EXPW == 0:
                        gpsimd.wait_ge(sem_ex, cm // EXPW + 1)
                        if cm >= NM:
                            gpsimd.wait_ge(sem_pl, cm - NM + 1)
                    if not mask_dve[cm]:
                        nc.gpsimd.tensor_scalar(
                            msk[cm % NM][:], iota_t[:],
                            bl_t[:, cm:cm + 1], expw_t[:, cm:cm + 1],
                            ALU.is_equal, ALU.mult,
                        ).then_inc(sem_mg, 1)

    return nc


def _plan(counts_k, n_banks):
    """Pass-1 chunking for one core: counts_k[j] = node count of local seg j.
    Returns per-group chunk lists [(jf, [(j, off, take), ...]), ...]."""
    groups = []
    nsegs = len(counts_k)
    for gb in range(n_banks):
        glo, ghi = 512 * gb, min(512 * (gb + 1), nsegs)
        chunks = []
        cur_nodes, cur_jf, cur_n = [], None, 0
        for j in range(glo, ghi):
            cnt = int(counts_k[j])
            off = 0
            while cnt > 0:
                if cur_jf is not None and j - cur_jf + 1 > KCAP:
                    chunks.append((cur_jf, cur_nodes))
                    cur_nodes, cur_jf, cur_n = [], None, 0
                if cur_jf is None:
                    cur_jf = j
                take = min(cnt, P - cur_n)
                cur_nodes.append((j, off, take))
                cur_n += take
                off += take
                cnt -= take
                if cur_n == P:
                    chunks.append((cur_jf, cur_nodes))
                    cur_nodes, cur_jf, cur_n = [], None, 0
        if cur_n > 0:
            chunks.append((cur_jf, cur_nodes))
        groups.append(chunks)
    return groups


def _pool(x, batch, W, num_graphs, n_cores=N_CORES):
    segs_per_core = num_graphs // n_cores
    n_banks = (segs_per_core + 511) // 512

    counts = np.bincount(batch, minlength=num_graphs).astype(np.int64)
    order = np.argsort(-counts, kind="stable")      # global seg ids, size desc
    orig_starts = np.zeros(num_graphs + 1, np.int64)
    np.cumsum(counts, out=orig_starts[1:])

    # snake deal: sorted position p -> (core, local j)
    nloc = num_graphs // n_cores
    pos = np.arange(num_graphs).reshape(nloc, n_cores)
    core_of_pos = np.where((np.arange(nloc) % 2 == 0)[:, None],
                           np.arange(n_cores)[None, :],
                           np.arange(n_cores)[None, :][:, ::-1])
    local_ids = np.empty((n_cores, nloc), np.int64)
    for k in range(n_cores):
        local_ids[k] = order[pos[core_of_pos == k]]
    local_counts = counts[local_ids]                # [n_cores, nloc]

    plans = [_plan(local_counts[k], n_banks) for k in range(n_cores)]
    ngc = [max(len(plans[k][g]) for k in range(n_cores)) for g in range(n_banks)]
    total = sum(ngc)
    ngc[-1] += (-total) % GRP
    nch = sum(ngc)

    G0 = [0]
    for t in ngc:
        G0.append(G0[-1] + t)
    W0 = np.full(nch, np.iinfo(np.int64).max, np.int64)
    W1 = np.full(nch, -1, np.int64)
    for k in range(n_cores):
        for g in range(n_banks):
            for i, (jf, nodes) in enumerate(plans[k][g]):
                c = G0[g] + i
                W0[c] = min(W0[c], jf)
                W1[c] = max(W1[c], nodes[-1][0])
    bank_of = np.empty(nch, np.int64)
    for g in range(n_banks):
        bank_of[G0[g]:G0[g + 1]] = g
        empt = W1[G0[g]:G0[g + 1]] < 0            # all-core-empty pad chunks
        W0[G0[g]:G0[g + 1]][empt] = 512 * g
        W1[G0[g]:G0[g + 1]][empt] = 512 * g
    K = int(max(2, (W1 - W0).max() + 1))
    jb_of = np.minimum(W0 - 512 * bank_of, 512 - K).astype(np.int64)
    assert jb_of.min() >= 0

    sup_sizes = [SUP_CH] * (nch // SUP_CH)
    if nch % SUP_CH:
        sup_sizes.append(nch % SUP_CH)

    # engine splits (tunable): masks on DVE (frac MD) else GPSIMD;
    # psum->sbuf copies: CPAT cycled over GRP-chunk groups (0=DVE, 1=ACT)
    mfrac = float(os.environ.get("MD", "0.3125"))
    mask_dve = [(int(c * mfrac) != int((c + 1) * mfrac)) for c in range(nch)]
    ngrp = nch // GRP
    cpat = [int(v) for v in os.environ.get("CPAT", "0,1").split(",")]
    copy_eng = [cpat[g % len(cpat)] for g in range(ngrp)]

    # per-core tensors
    x_bf = np.ascontiguousarray(x).astype(ml_dtypes.bfloat16)
    in_maps, pad_counts = [], []
    for k in range(n_cores):
        xflat = np.zeros((nch * P, D), ml_dtypes.bfloat16)
        blflat = np.full((nch * P,), 999.0, np.float32)
        real = 0
        for g in range(n_banks):
            for i, (jf, nodes) in enumerate(plans[k][g]):
                c = G0[g] + i
                base = 512 * bank_of[c] + jb_of[c]
                p0 = c * P
                for (j, off, take) in nodes:
                    gid = local_ids[k][j]
                    s0 = orig_starts[gid] + off
                    xflat[p0:p0 + take] = x_bf[s0:s0 + take]
                    blflat[p0:p0 + take] = j - base
                    p0 += take
                    real += take
        pad_counts.append(nch * P - real)
        # slab per super: (c, n, d) -> (d, c, n)
        slabs = []
        o = 0
        for ch in sup_sizes:
            a = xflat[o * P:(o + ch) * P]
            slabs.append(np.ascontiguousarray(
                a.reshape(ch, P, D).transpose(2, 0, 1)).reshape(-1))
            o += ch
        xp = np.concatenate(slabs)
        bl = np.ascontiguousarray(blflat.reshape(nch, P).T).astype(np.float32)
        in_maps.append({
            "xp": xp, "bl": bl,
            "wcol": np.asarray(W, np.float32).reshape(P, 1).astype(ml_dtypes.bfloat16),
            "ident": np.eye(P, dtype=ml_dtypes.bfloat16),
            "iota": np.broadcast_to(
                np.arange(K).astype(ml_dtypes.bfloat16), (P, K)).copy(),
        })

    key = hashlib.sha1(
        np.concatenate([bank_of, jb_of, [nch, K, n_banks]]).tobytes()
        + bytes(mask_dve) + bytes(copy_eng) + bytes(str(sup_sizes), "ascii")
    ).hexdigest()
    if key not in _prog_cache:
        _prog_cache[key] = _build(nch, K, n_banks, bank_of.tolist(),
                                  jb_of.tolist(), sup_sizes, mask_dve, copy_eng)
    nc = _prog_cache[key]

    res = run_bass_kernel_spmd(nc, in_maps, list(range(n_cores))).results

    z_total = 0.0
    out = np.zeros((num_graphs, D), np.float32)
    for k in range(n_cores):
        z_total += float(res[k]["zout"].astype(np.float64).sum()) - pad_counts[k]
        o = res[k]["outp"].astype(np.float32)       # [D, n_banks*512]
        out[local_ids[k]] = o.T[:nloc]
    return (out / np.float32(z_total)).astype(np.float32)


def kernel(x, batch, W, b):
    x = np.asarray(x, np.float32)
    batch = np.asarray(batch).astype(np.int64)
    W = np.asarray(W, np.float32)
    return _pool(x, batch, W, num_graphs=16384)


if __name__ == "__main__":
    rng = np.random.default_rng(0)
    G = int(os.environ.get("TG", "1024"))
    n = int(os.environ.get("TN", "64000"))
    x = rng.standard_normal((n, D), dtype=np.float32)
    batch = np.sort(rng.integers(0, G, n)).astype(np.int64)
    W = (rng.standard_normal((D, 1), dtype=np.float32) / np.sqrt(D)).astype(np.float32)
    b = np.zeros((1,), np.float32)

    got = _pool(x, batch, W, num_graphs=G)

    s = (x @ W).ravel()
    a = np.exp(s - s.max()); a /= a.sum()
    want = np.zeros((G, D), np.float64)
    np.add.at(want, batch, x * a[:, None])
    want = want.astype(np.float32)
    num = np.abs(got - want).max()
    print("abs err:", num, "rel err:", num / np.abs(want).max())


# revision 21
# speedup vs baseline: 1.8507x; 1.0141x over previous
"""AttentionPooling (global-softmax segment-sum) Trainium2 Bass kernel.

  scores = x @ W + b ; attn = softmax(scores, axis=0) ; out = segment_sum(x*attn, batch, G)

Design (8 cores, SPMD, raw Bass; softmax is shift-invariant so b drops out and
the fixed shift is 0; device computes unnormalized pooled sums + Z partials,
host divides at the end):

 * Segments are sorted by size (desc) and snake-dealt to the 8 cores, so every
   core sees a near-identical segment-size profile (cumulative node drift
   between cores < 1 chunk).  That allows ONE shared SPMD program in which
   chunk c of every core covers segments inside a shared window
   [W0(c), W0(c)+K) with small K (~4): the segment-scatter matrix per chunk is
   only [128, K] instead of a full [128, 128] one-hot.
 * x ships TRANSPOSED per 128-node chunk: xT_c [d=128 part, n=128 free] bf16,
   packed in 32-chunk DMA slabs (8 KB/partition lines -> full DMA efficiency).
 * PE per chunk (matmul operands in SBUF):
     scores:   mm(lhsT=xT_c, rhs=W[d,1])   -> psum col  [n,1] f32   (~2 ns)
     untrans:  PE transpose(xT_c)          -> psum x_c [n,d] BF16   (~53 ns)
     pooled:   mm(lhsT=x_c(sbuf), rhs=M_c[n,K]) += psum out[d, segcols]
               (start=False, banks double-buffered)                 (~2 ns)
   The pooled output lands TRANSPOSED [d, seg]; the host untransposes.
 * bf16 psum transposes pack 8 chunks per bank, so the psum->sbuf copies are
   [128, 1024] bf16 ops (2x mode on DVE) split across DVE and ACT.
 * ACT: Exp on 32-wide score strips (psum f32 -> sbuf expw f32).
 * DVE/GP: masks M_c = (iota_K == bl_c) * expw_c via one tensor_scalar
   [128,K] bf16 per chunk (~61/99 ns).  Z = one tensor_reduce over expw
   [128, nch] at the very end.

PSUM hazard rule (found the hard way; the device hangs otherwise): a bank PE
is writing must not be concurrently accessed by ACT/DVE.  Hence: scores
alternate between 2 banks per 32-chunk strip and PE re-enters a parity only
after that parity's previous Exp finished; transposed x rotates 4 banks
(copies read banks PE is not writing); the out accumulators are 2 banks
double-buffered over the (chunk-sequential) 512-segment ranges with
flush+memset strictly between PE uses.

TimelineSim (the graded cost model) is DMA-bound: the 65 MB/core bf16 x
stream at the modeled 360 GB/s is ~184 us.
"""

import hashlib
import os
import numpy as np
import ml_dtypes

import concourse.bass as bass
import concourse.mybir as mybir
from concourse.bass_utils import run_bass_kernel_spmd

BF16 = mybir.dt.bfloat16
F32 = mybir.dt.float32
ALU = mybir.AluOpType
ACTF = mybir.ActivationFunctionType

N_CORES = 8
P = 128
D = 128
SUP_CH = 32          # chunks per DMA super-slab
GRP = 8              # chunks per transpose-psum bank / copy op
NXB = 10             # xT slab ring depth
NT = 4               # transpose psum bank rotation
NXS = 12             # copied-back x_c sbuf slots (GRP-chunk groups)
NM = 128             # mask sbuf slots
LAG_G = 9            # pooled mms lag transposes by this many GRP-groups
EXPW = 32            # chunks per Exp strip
MLAG = 16            # exp/masks lag copies by this many chunks
KCAP = 16            # pass-1 span cap

_prog_cache = {}


def _build(nch, K, n_banks, bank_of, jb_of, sup_sizes, mask_dve, copy_eng):
    """Shared SPMD program.  bank_of/jb_of: per-chunk out range and column
    base.  sup_sizes: chunks per DMA super.  mask_dve[c]: mask built on DVE
    (else GPSIMD).  copy_eng[g]: 0=DVE 1=ACT for GRP-chunk psum->sbuf copies."""
    nsup = len(sup_sizes)
    CH0 = [0]
    for t in sup_sizes:
        CH0.append(CH0[-1] + t)
    assert CH0[-1] == nch and nch % GRP == 0
    ngrp = nch // GRP
    n_exp = (nch + EXPW - 1) // EXPW
    sup_of = []
    for s in range(nsup):
        sup_of += [s] * sup_sizes[s]

    # cumulative ticks
    mskd_tick = np.cumsum(mask_dve).tolist()
    mskg_tick = np.cumsum([not m for m in mask_dve]).tolist()
    cp_tick = [0] * ngrp
    cnt = [0, 0]
    for g in range(ngrp):
        cnt[copy_eng[g]] += 1
        cp_tick[g] = cnt[copy_eng[g]]

    G0 = [0]
    for c in range(nch):
        if bank_of[c] != len(G0) - 1:
            G0.append(c)
    G0 += [nch] * (n_banks + 1 - len(G0))

    nc = bass.Bass()
    xp_h = nc.declare_dram_parameter("xp", [nch * P * D], BF16, isOutput=False)
    bl_h = nc.declare_dram_parameter("bl", [P, nch], F32, isOutput=False)
    wcol_h = nc.declare_dram_parameter("wcol", [P, 1], BF16, isOutput=False)
    ident_h = nc.declare_dram_parameter("ident", [P, P], BF16, isOutput=False)
    iota_h = nc.declare_dram_parameter("iota", [P, K], BF16, isOutput=False)
    out_h = nc.declare_dram_parameter("outp", [P, n_banks * 512], F32, isOutput=True)
    z_h = nc.declare_dram_parameter("zout", [P, 1], F32, isOutput=True)

    import contextlib
    with contextlib.ExitStack() as ctx:
        sem_x = [ctx.enter_context(nc.semaphore(f"sem_x{j}")) for j in range(NXB)]
        sem_cst = ctx.enter_context(nc.semaphore("sem_cst"))
        sem_sc = ctx.enter_context(nc.semaphore("sem_sc"))
        sem_tr = ctx.enter_context(nc.semaphore("sem_tr"))
        sem_ex = ctx.enter_context(nc.semaphore("sem_ex"))
        sem_md = ctx.enter_context(nc.semaphore("sem_md"))
        sem_mg = ctx.enter_context(nc.semaphore("sem_mg"))
        sem_pl = ctx.enter_context(nc.semaphore("sem_pl"))
        sem_cp = [ctx.enter_context(nc.semaphore(f"sem_cp{e}")) for e in range(2)]
        sem_ini = ctx.enter_context(nc.semaphore("sem_ini"))
        sem_zr = ctx.enter_context(nc.semaphore("sem_zr"))
        sem_fl = ctx.enter_context(nc.semaphore("sem_fl"))
        sem_out = ctx.enter_context(nc.semaphore("sem_out"))

        xt = [ctx.enter_context(nc.sbuf_tensor(f"xt{j}", [P, SUP_CH * D], BF16))
              for j in range(NXB)]
        wcol_t = ctx.enter_context(nc.sbuf_tensor("wcol_t", [P, 1], BF16))
        ident_t = ctx.enter_context(nc.sbuf_tensor("ident_t", [P, P], BF16))
        iota_t = ctx.enter_context(nc.sbuf_tensor("iota_t", [P, K], BF16))
        bl_t = ctx.enter_context(nc.sbuf_tensor("bl_t", [P, nch], F32))
        expw_t = ctx.enter_context(nc.sbuf_tensor("expw_t", [P, nch], F32))
        xsb = [ctx.enter_context(nc.sbuf_tensor(f"xsb{j}", [P, GRP * D], BF16))
               for j in range(NXS)]
        msk = [ctx.enter_context(nc.sbuf_tensor(f"msk{j}", [P, K], BF16))
               for j in range(NM)]
        stage_t = ctx.enter_context(nc.sbuf_tensor("stage_t", [P, n_banks * 512], F32))
        zsum_t = ctx.enter_context(nc.sbuf_tensor("zsum_t", [P, 1], F32))

        # PSUM hazard rule: a bank PE is writing must never be concurrently
        # accessed by ACT/DVE (the device hangs).  Scores: 2 banks alternated
        # per strip; transposes: 4 bf16 banks of GRP chunks; out: 2 banks
        # double-buffered over the sequential 512-seg ranges.
        sp2 = [ctx.enter_context(nc.psum_tensor(f"sp{i}", [P, 512], F32))
               for i in range(2)]
        tp = [ctx.enter_context(nc.psum_tensor(f"tp{j}", [P, GRP * D], BF16))
              for j in range(NT)]
        outp2 = [ctx.enter_context(nc.psum_tensor(f"op{b}", [P, 512], F32))
                 for b in range(2)]

        N_CST = 4  # preamble DMAs

        def sploc(c):
            e = c // EXPW
            return sp2[e % 2], ((e // 2) * EXPW) % 512 + (c % EXPW)

        def pooled_group(tensor, go, tail):
            c0 = GRP * go
            if c0 % EXPW == 0:
                ce = min(c0 + EXPW, nch) - 1
                tensor.wait_ge(sem_md, mskd_tick[ce])
                tensor.wait_ge(sem_mg, mskg_tick[ce])
            if tail and go >= ngrp - NT:
                tensor.wait_ge(sem_cp[copy_eng[go]], cp_tick[go])
            for cc in range(c0, c0 + GRP):
                r = bank_of[cc]
                if r >= 2 and cc == G0[r]:
                    tensor.wait_ge(sem_ini, r + 1)   # memset of reused bank
                nc.tensor.matmul(
                    outp2[r % 2][:, jb_of[cc]:jb_of[cc] + K],
                    lhsT=xsb[go % NXS][:, (cc % GRP) * D:(cc % GRP + 1) * D],
                    rhs=msk[cc % NM][:],
                    start=False, stop=True, skip_group_check=True,
                ).then_inc(sem_pl, 1)

        with nc.Block() as block:

            @block.sync
            def _(sync):
                sync.dma_start(out=wcol_t[:], in_=wcol_h[:]).then_inc(sem_cst, 16)
                sync.dma_start(out=ident_t[:], in_=ident_h[:]).then_inc(sem_cst, 16)
                sync.dma_start(out=iota_t[:], in_=iota_h[:]).then_inc(sem_cst, 16)
                sync.dma_start(out=bl_t[:], in_=bl_h[:]).then_inc(sem_cst, 16)
                for s in range(nsup):
                    j = s % NXB
                    ch = sup_sizes[s]
                    if s >= NXB:
                        sync.wait_ge(sem_tr, CH0[s - NXB + 1])
                    sync.dma_start(
                        out=xt[j][:, 0:ch * D],
                        in_=xp_h[CH0[s] * P * D:CH0[s + 1] * P * D].rearrange(
                            "(d f) -> d f", d=P),
                    ).then_inc(sem_x[j], 16)
                for b in range(n_banks):
                    sync.wait_ge(sem_fl, b + 1)
                    sync.dma_start(
                        out=out_h[:, b * 512:(b + 1) * 512],
                        in_=stage_t[:, b * 512:(b + 1) * 512],
                    ).then_inc(sem_out, 16)
                sync.wait_ge(sem_zr, 1)
                sync.dma_start(out=z_h[:], in_=zsum_t[:]).then_inc(sem_out, 16)
                sync.wait_ge(sem_out, 16 * (n_banks + 1))

            @block.tensor
            def _(tensor):
                tensor.wait_ge(sem_cst, 16 * N_CST)
                tensor.wait_ge(sem_ini, 2)
                for c in range(nch):
                    s = sup_of[c]
                    ci = c - CH0[s]
                    if ci == 0:
                        tensor.wait_ge(sem_x[s % NXB], 16 * (s // NXB + 1))
                    if c % EXPW == 0 and c // EXPW >= 2:
                        # reuse of this parity's score bank: prior strip's Exp
                        tensor.wait_ge(sem_ex, c // EXPW - 1)
                    xsl = xt[s % NXB][:, ci * D:(ci + 1) * D]
                    bnk, col = sploc(c)
                    nc.tensor.matmul(
                        bnk[:, col:col + 1],
                        lhsT=xsl, rhs=wcol_t[:],
                        start=True, stop=True, skip_group_check=True,
                    ).then_inc(sem_sc, 1)
                    g = c // GRP
                    if c % GRP == 0 and g >= NT:
                        go2 = g - NT
                        tensor.wait_ge(sem_cp[copy_eng[go2]], cp_tick[go2])
                    nc.tensor.transpose(
                        tp[g % NT][:, (c % GRP) * D:(c % GRP + 1) * D],
                        xsl, ident_t[:],
                    ).then_inc(sem_tr, 1)
                    if c % GRP == GRP - 1 and g >= LAG_G:
                        pooled_group(tensor, g - LAG_G, False)
                for go in range(max(0, ngrp - LAG_G), ngrp):
                    pooled_group(tensor, go, True)

            # Copies run at position p; exp/masks trail at p-MLAG so neither
            # ACT nor DVE blocks on exp before emitting a copy PE waits on.

            @block.scalar
            def _(scalar):
                nfl = 0
                for p in range(0, nch + MLAG, GRP):
                    g = p // GRP
                    if g < ngrp and copy_eng[g] == 1:
                        scalar.wait_ge(sem_tr, GRP * g + GRP)
                        if g >= NXS:
                            scalar.wait_ge(sem_pl, GRP * (g - NXS) + GRP)
                        nc.scalar.copy(
                            out=xsb[g % NXS][:], in_=tp[g % NT][:],
                        ).then_inc(sem_cp[1], 1)
                    cm = p - MLAG
                    if cm >= 0 and cm % EXPW == 0:
                        e = cm // EXPW
                        c0, c1 = EXPW * e, min(EXPW * e + EXPW, nch)
                        scalar.wait_ge(sem_sc, c1)
                        bnk, col = sploc(c0)
                        nc.scalar.activation(
                            out=expw_t[:, c0:c1],
                            in_=bnk[:, col:col + (c1 - c0)],
                            func=ACTF.Exp,
                        ).then_inc(sem_ex, 1)
                    while nfl < n_banks and G0[nfl + 1] + 80 <= p:
                        scalar.wait_ge(sem_pl, G0[nfl + 1])
                        nc.scalar.copy(
                            out=stage_t[:, nfl * 512:(nfl + 1) * 512],
                            in_=outp2[nfl % 2][:],
                        ).then_inc(sem_fl, 1)
                        nfl += 1
                while nfl < n_banks:
                    scalar.wait_ge(sem_pl, G0[nfl + 1])
                    nc.scalar.copy(
                        out=stage_t[:, nfl * 512:(nfl + 1) * 512],
                        in_=outp2[nfl % 2][:],
                    ).then_inc(sem_fl, 1)
                    nfl += 1

            @block.vector
            def _(vector):
                for b in range(2):
                    nc.vector.memset(outp2[b][:], 0.0).then_inc(sem_ini, 1)
                vector.wait_ge(sem_cst, 16 * N_CST)
                nms = 2
                for p in range(nch + MLAG):
                    g = p // GRP
                    if p < nch and p % GRP == GRP - 1 and copy_eng[g] == 0:
                        vector.wait_ge(sem_tr, GRP * g + GRP)
                        if g >= NXS:
                            vector.wait_ge(sem_pl, GRP * (g - NXS) + GRP)
                        nc.vector.tensor_copy(
                            out=xsb[g % NXS][:], in_=tp[g % NT][:],
                        ).then_inc(sem_cp[0], 1)
                    while nms < n_banks and G0[nms - 1] + 96 <= p:
                        vector.wait_ge(sem_fl, nms - 1)
                        nc.vector.memset(outp2[nms % 2][:], 0.0).then_inc(sem_ini, 1)
                        nms += 1
                    cm = p - MLAG
                    if cm < 0:
                        continue
                    if cm % EXPW == 0:
                        vector.wait_ge(sem_ex, cm // EXPW + 1)
                        if cm >= NM:
                            vector.wait_ge(sem_pl, cm - NM + 1)
                    if mask_dve[cm]:
                        nc.vector.tensor_scalar(
                            msk[cm % NM][:], iota_t[:],
                            bl_t[:, cm:cm + 1], expw_t[:, cm:cm + 1],
                            ALU.is_equal, ALU.mult,
                        ).then_inc(sem_md, 1)
                # Z = sum over all chunks of expw (pads contribute e^0=1
                # each; host subtracts the pad count)
                vector.wait_ge(sem_ex, n_exp)
                nc.vector.tensor_reduce(
                    out=zsum_t[:], in_=expw_t[:],
                    axis=mybir.AxisListType.X, op=ALU.add,
                ).then_inc(sem_zr, 1)

            @block.gpsimd
            def _(gpsimd):
                # GPSIMD cannot access PSUM: masks only
                gpsimd.wait_ge(sem_cst, 16 * N_CST)
                for cm in range(nch):
                    if cm % EXPW == 0:
                        gpsimd.wait_ge(sem_ex, cm // EXPW + 1)
                        if cm >= NM:
                            gpsimd.wait_ge(sem_pl, cm - NM + 1)
                    if not mask_dve[cm]:
                        nc.gpsimd.tensor_scalar(
                            msk[cm % NM][:], iota_t[:],
                            bl_t[:, cm:cm + 1], expw_t[:, cm:cm + 1],
                            ALU.is_equal, ALU.mult,
                        ).then_inc(sem_mg, 1)

    return nc


def _plan(counts_k, n_banks):
    """Pass-1 chunking for one core: counts_k[j] = node count of local seg j.
    Returns per-group chunk lists [(jf, [(j, off, take), ...]), ...]."""
    groups = []
    nsegs = len(counts_k)
    for gb in range(n_banks):
        glo, ghi = 512 * gb, min(512 * (gb + 1), nsegs)
        chunks = []
        cur_nodes, cur_jf, cur_n = [], None, 0
        for j in range(glo, ghi):
            cnt = int(counts_k[j])
            off = 0
            while cnt > 0:
                if cur_jf is not None and j - cur_jf + 1 > KCAP:
                    chunks.append((cur_jf, cur_nodes))
                    cur_nodes, cur_jf, cur_n = [], None, 0
                if cur_jf is None:
                    cur_jf = j
                take = min(cnt, P - cur_n)
                cur_nodes.append((j, off, take))
                cur_n += take
                off += take
                cnt -= take
                if cur_n == P:
                    chunks.append((cur_jf, cur_nodes))
                    cur_nodes, cur_jf, cur_n = [], None, 0
        if cur_n > 0:
            chunks.append((cur_jf, cur_nodes))
        groups.append(chunks)
    return groups


def _pool(x, batch, W, num_graphs, n_cores=N_CORES):
    segs_per_core = num_graphs // n_cores
    n_banks = (segs_per_core + 511) // 512

    counts = np.bincount(batch, minlength=num_graphs).astype(np.int64)
    order = np.argsort(-counts, kind="stable")      # global seg ids, size desc
    orig_starts = np.zeros(num_graphs + 1, np.int64)
    np.cumsum(counts, out=orig_starts[1:])

    # snake deal: sorted position p -> (core, local j)
    nloc = num_graphs // n_cores
    pos = np.arange(num_graphs).reshape(nloc, n_cores)
    core_of_pos = np.where((np.arange(nloc) % 2 == 0)[:, None],
                           np.arange(n_cores)[None, :],
                           np.arange(n_cores)[None, :][:, ::-1])
    local_ids = np.empty((n_cores, nloc), np.int64)
    for k in range(n_cores):
        local_ids[k] = order[pos[core_of_pos == k]]
    local_counts = counts[local_ids]                # [n_cores, nloc]

    plans = [_plan(local_counts[k], n_banks) for k in range(n_cores)]
    ngc = [max(len(plans[k][g]) for k in range(n_cores)) for g in range(n_banks)]
    total = sum(ngc)
    ngc[-1] += (-total) % GRP
    nch = sum(ngc)

    G0 = [0]
    for t in ngc:
        G0.append(G0[-1] + t)
    W0 = np.full(nch, np.iinfo(np.int64).max, np.int64)
    W1 = np.full(nch, -1, np.int64)
    for k in range(n_cores):
        for g in range(n_banks):
            for i, (jf, nodes) in enumerate(plans[k][g]):
                c = G0[g] + i
                W0[c] = min(W0[c], jf)
                W1[c] = max(W1[c], nodes[-1][0])
    bank_of = np.empty(nch, np.int64)
    for g in range(n_banks):
        bank_of[G0[g]:G0[g + 1]] = g
        empt = W1[G0[g]:G0[g + 1]] < 0            # all-core-empty pad chunks
        W0[G0[g]:G0[g + 1]][empt] = 512 * g
        W1[G0[g]:G0[g + 1]][empt] = 512 * g
    K = int(max(2, (W1 - W0).max() + 1))
    jb_of = np.minimum(W0 - 512 * bank_of, 512 - K).astype(np.int64)
    assert jb_of.min() >= 0

    sup_sizes = [SUP_CH] * (nch // SUP_CH)
    if nch % SUP_CH:
        sup_sizes.append(nch % SUP_CH)

    # engine splits (tunable): masks on DVE (frac MD) else GPSIMD;
    # psum->sbuf copies: CPAT cycled over GRP-chunk groups (0=DVE, 1=ACT)
    mfrac = float(os.environ.get("MD", "0.3125"))
    mask_dve = [(int(c * mfrac) != int((c + 1) * mfrac)) for c in range(nch)]
    for c in range(max(0, nch - 64), nch):   # parallelize the drain tail
        mask_dve[c] = (c % 2 == 0)
    ngrp = nch // GRP
    cpat = [int(v) for v in os.environ.get("CPAT", "0,1").split(",")]
    copy_eng = [cpat[g % len(cpat)] for g in range(ngrp)]

    # per-core tensors
    x_bf = np.ascontiguousarray(x).astype(ml_dtypes.bfloat16)
    in_maps, pad_counts = [], []
    for k in range(n_cores):
        xflat = np.zeros((nch * P, D), ml_dtypes.bfloat16)
        blflat = np.full((nch * P,), 999.0, np.float32)
        real = 0
        for g in range(n_banks):
            for i, (jf, nodes) in enumerate(plans[k][g]):
                c = G0[g] + i
                base = 512 * bank_of[c] + jb_of[c]
                p0 = c * P
                for (j, off, take) in nodes:
                    gid = local_ids[k][j]
                    s0 = orig_starts[gid] + off
                    xflat[p0:p0 + take] = x_bf[s0:s0 + take]
                    blflat[p0:p0 + take] = j - base
                    p0 += take
                    real += take
        pad_counts.append(nch * P - real)
        # slab per super: (c, n, d) -> (d, c, n)
        slabs = []
        o = 0
        for ch in sup_sizes:
            a = xflat[o * P:(o + ch) * P]
            slabs.append(np.ascontiguousarray(
                a.reshape(ch, P, D).transpose(2, 0, 1)).reshape(-1))
            o += ch
        xp = np.concatenate(slabs)
        bl = np.ascontiguousarray(blflat.reshape(nch, P).T).astype(np.float32)
        in_maps.append({
            "xp": xp, "bl": bl,
            "wcol": np.asarray(W, np.float32).reshape(P, 1).astype(ml_dtypes.bfloat16),
            "ident": np.eye(P, dtype=ml_dtypes.bfloat16),
            "iota": np.broadcast_to(
                np.arange(K).astype(ml_dtypes.bfloat16), (P, K)).copy(),
        })

    key = hashlib.sha1(
        np.concatenate([bank_of, jb_of, [nch, K, n_banks]]).tobytes()
        + bytes(mask_dve) + bytes(copy_eng) + bytes(str(sup_sizes), "ascii")
    ).hexdigest()
    if key not in _prog_cache:
        _prog_cache[key] = _build(nch, K, n_banks, bank_of.tolist(),
                                  jb_of.tolist(), sup_sizes, mask_dve, copy_eng)
    nc = _prog_cache[key]

    res = run_bass_kernel_spmd(nc, in_maps, list(range(n_cores))).results

    z_total = 0.0
    out = np.zeros((num_graphs, D), np.float32)
    for k in range(n_cores):
        z_total += float(res[k]["zout"].astype(np.float64).sum()) - pad_counts[k]
        o = res[k]["outp"].astype(np.float32)       # [D, n_banks*512]
        out[local_ids[k]] = o.T[:nloc]
    return (out / np.float32(z_total)).astype(np.float32)


def kernel(x, batch, W, b):
    x = np.asarray(x, np.float32)
    batch = np.asarray(batch).astype(np.int64)
    W = np.asarray(W, np.float32)
    return _pool(x, batch, W, num_graphs=16384)


if __name__ == "__main__":
    rng = np.random.default_rng(0)
    G = int(os.environ.get("TG", "1024"))
    n = int(os.environ.get("TN", "64000"))
    x = rng.standard_normal((n, D), dtype=np.float32)
    batch = np.sort(rng.integers(0, G, n)).astype(np.int64)
    W = (rng.standard_normal((D, 1), dtype=np.float32) / np.sqrt(D)).astype(np.float32)
    b = np.zeros((1,), np.float32)

    got = _pool(x, batch, W, num_graphs=G)

    s = (x @ W).ravel()
    a = np.exp(s - s.max()); a /= a.sum()
    want = np.zeros((G, D), np.float64)
    np.add.at(want, batch, x * a[:, None])
    want = want.astype(np.float32)
    num = np.abs(got - want).max()
    print("abs err:", num, "rel err:", num / np.abs(want).max())


# revision 22
# speedup vs baseline: 1.8543x; 1.0020x over previous
"""AttentionPooling (global-softmax segment-sum) Trainium2 Bass kernel.

  scores = x @ W + b ; attn = softmax(scores, axis=0) ; out = segment_sum(x*attn, batch, G)

Design (8 cores, SPMD, raw Bass; softmax is shift-invariant so b drops out and
the fixed shift is 0; device computes unnormalized pooled sums + Z partials,
host divides at the end):

 * Segments are sorted by size (desc) and snake-dealt to the 8 cores, so every
   core sees a near-identical segment-size profile (cumulative node drift
   between cores < 1 chunk).  That allows ONE shared SPMD program in which
   chunk c of every core covers segments inside a shared window
   [W0(c), W0(c)+K) with small K (~4): the segment-scatter matrix per chunk is
   only [128, K] instead of a full [128, 128] one-hot.
 * x ships TRANSPOSED per 128-node chunk: xT_c [d=128 part, n=128 free] bf16,
   packed in 32-chunk DMA slabs (8 KB/partition lines -> full DMA efficiency).
 * PE per chunk (matmul operands in SBUF):
     scores:   mm(lhsT=xT_c, rhs=W[d,1])   -> psum col  [n,1] f32   (~2 ns)
     untrans:  PE transpose(xT_c)          -> psum x_c [n,d] BF16   (~53 ns)
     pooled:   mm(lhsT=x_c(sbuf), rhs=M_c[n,K]) += psum out[d, segcols]
               (start=False, banks double-buffered)                 (~2 ns)
   The pooled output lands TRANSPOSED [d, seg]; the host untransposes.
 * bf16 psum transposes pack 8 chunks per bank, so the psum->sbuf copies are
   [128, 1024] bf16 ops (2x mode on DVE) split across DVE and ACT.
 * ACT: Exp on 32-wide score strips (psum f32 -> sbuf expw f32).
 * DVE/GP: masks M_c = (iota_K == bl_c) * expw_c via one tensor_scalar
   [128,K] bf16 per chunk (~61/99 ns).  Z = one tensor_reduce over expw
   [128, nch] at the very end.

PSUM hazard rule (found the hard way; the device hangs otherwise): a bank PE
is writing must not be concurrently accessed by ACT/DVE.  Hence: scores
alternate between 2 banks per 32-chunk strip and PE re-enters a parity only
after that parity's previous Exp finished; transposed x rotates 4 banks
(copies read banks PE is not writing); the out accumulators are 2 banks
double-buffered over the (chunk-sequential) 512-segment ranges with
flush+memset strictly between PE uses.

TimelineSim (the graded cost model) is DMA-bound: the 65 MB/core bf16 x
stream at the modeled 360 GB/s is ~184 us.
"""

import hashlib
import os
import numpy as np
import ml_dtypes

import concourse.bass as bass
import concourse.mybir as mybir
from concourse.bass_utils import run_bass_kernel_spmd

BF16 = mybir.dt.bfloat16
F32 = mybir.dt.float32
ALU = mybir.AluOpType
ACTF = mybir.ActivationFunctionType

N_CORES = 8
P = 128
D = 128
SUP_CH = 32          # chunks per DMA super-slab
GRP = 8              # chunks per transpose-psum bank / copy op
NXB = 10             # xT slab ring depth
NT = 4               # transpose psum bank rotation
NXS = 12             # copied-back x_c sbuf slots (GRP-chunk groups)
NM = 128             # mask sbuf slots
LAG_G = 9            # pooled mms lag transposes by this many GRP-groups
EXPW = 32            # chunks per Exp strip
MLAG = 16            # exp/masks lag copies by this many chunks
KCAP = 16            # pass-1 span cap

_prog_cache = {}


def _build(nch, K, n_banks, bank_of, jb_of, sup_sizes, mask_dve, copy_eng):
    """Shared SPMD program.  bank_of/jb_of: per-chunk out range and column
    base.  sup_sizes: chunks per DMA super.  mask_dve[c]: mask built on DVE
    (else GPSIMD).  copy_eng[g]: 0=DVE 1=ACT for GRP-chunk psum->sbuf copies."""
    nsup = len(sup_sizes)
    CH0 = [0]
    for t in sup_sizes:
        CH0.append(CH0[-1] + t)
    assert CH0[-1] == nch and nch % GRP == 0
    ngrp = nch // GRP
    n_exp = (nch + EXPW - 1) // EXPW
    sup_of = []
    for s in range(nsup):
        sup_of += [s] * sup_sizes[s]

    # cumulative ticks
    mskd_tick = np.cumsum(mask_dve).tolist()
    mskg_tick = np.cumsum([not m for m in mask_dve]).tolist()
    cp_tick = [0] * ngrp
    cnt = [0, 0]
    for g in range(ngrp):
        cnt[copy_eng[g]] += 1
        cp_tick[g] = cnt[copy_eng[g]]

    G0 = [0]
    for c in range(nch):
        if bank_of[c] != len(G0) - 1:
            G0.append(c)
    G0 += [nch] * (n_banks + 1 - len(G0))

    nc = bass.Bass()
    xp_h = nc.declare_dram_parameter("xp", [nch * P * D], BF16, isOutput=False)
    bl_h = nc.declare_dram_parameter("bl", [P, nch], F32, isOutput=False)
    wcol_h = nc.declare_dram_parameter("wcol", [P, 1], BF16, isOutput=False)
    ident_h = nc.declare_dram_parameter("ident", [P, P], BF16, isOutput=False)
    iota_h = nc.declare_dram_parameter("iota", [P, K], BF16, isOutput=False)
    out_h = nc.declare_dram_parameter("outp", [P, n_banks * 512], F32, isOutput=True)
    z_h = nc.declare_dram_parameter("zout", [P, 1], F32, isOutput=True)

    import contextlib
    with contextlib.ExitStack() as ctx:
        sem_x = [ctx.enter_context(nc.semaphore(f"sem_x{j}")) for j in range(NXB)]
        sem_cst = ctx.enter_context(nc.semaphore("sem_cst"))
        sem_sc = ctx.enter_context(nc.semaphore("sem_sc"))
        sem_tr = ctx.enter_context(nc.semaphore("sem_tr"))
        sem_ex = ctx.enter_context(nc.semaphore("sem_ex"))
        sem_md = ctx.enter_context(nc.semaphore("sem_md"))
        sem_mg = ctx.enter_context(nc.semaphore("sem_mg"))
        sem_pl = ctx.enter_context(nc.semaphore("sem_pl"))
        sem_cp = [ctx.enter_context(nc.semaphore(f"sem_cp{e}")) for e in range(2)]
        sem_ini = ctx.enter_context(nc.semaphore("sem_ini"))
        sem_zr = ctx.enter_context(nc.semaphore("sem_zr"))
        sem_fl = ctx.enter_context(nc.semaphore("sem_fl"))
        sem_out = ctx.enter_context(nc.semaphore("sem_out"))

        xt = [ctx.enter_context(nc.sbuf_tensor(f"xt{j}", [P, SUP_CH * D], BF16))
              for j in range(NXB)]
        wcol_t = ctx.enter_context(nc.sbuf_tensor("wcol_t", [P, 1], BF16))
        ident_t = ctx.enter_context(nc.sbuf_tensor("ident_t", [P, P], BF16))
        iota_t = ctx.enter_context(nc.sbuf_tensor("iota_t", [P, K], BF16))
        bl_t = ctx.enter_context(nc.sbuf_tensor("bl_t", [P, nch], F32))
        expw_t = ctx.enter_context(nc.sbuf_tensor("expw_t", [P, nch], F32))
        xsb = [ctx.enter_context(nc.sbuf_tensor(f"xsb{j}", [P, GRP * D], BF16))
               for j in range(NXS)]
        msk = [ctx.enter_context(nc.sbuf_tensor(f"msk{j}", [P, K], BF16))
               for j in range(NM)]
        stage_t = ctx.enter_context(nc.sbuf_tensor("stage_t", [P, n_banks * 512], F32))
        zsum_t = ctx.enter_context(nc.sbuf_tensor("zsum_t", [P, 1], F32))

        # PSUM hazard rule: a bank PE is writing must never be concurrently
        # accessed by ACT/DVE (the device hangs).  Scores: 2 banks alternated
        # per strip; transposes: 4 bf16 banks of GRP chunks; out: 2 banks
        # double-buffered over the sequential 512-seg ranges.
        sp2 = [ctx.enter_context(nc.psum_tensor(f"sp{i}", [P, 512], F32))
               for i in range(2)]
        tp = [ctx.enter_context(nc.psum_tensor(f"tp{j}", [P, GRP * D], BF16))
              for j in range(NT)]
        outp2 = [ctx.enter_context(nc.psum_tensor(f"op{b}", [P, 512], F32))
                 for b in range(2)]

        N_CST = 4  # preamble DMAs

        def sploc(c):
            e = c // EXPW
            return sp2[e % 2], ((e // 2) * EXPW) % 512 + (c % EXPW)

        def pooled_group(tensor, go, tail):
            c0 = GRP * go
            if c0 % EXPW == 0:
                ce = min(c0 + EXPW, nch) - 1
                tensor.wait_ge(sem_md, mskd_tick[ce])
                tensor.wait_ge(sem_mg, mskg_tick[ce])
            if tail and go >= ngrp - NT:
                tensor.wait_ge(sem_cp[copy_eng[go]], cp_tick[go])
            for cc in range(c0, c0 + GRP):
                r = bank_of[cc]
                if r >= 2 and cc == G0[r]:
                    tensor.wait_ge(sem_ini, r + 1)   # memset of reused bank
                nc.tensor.matmul(
                    outp2[r % 2][:, jb_of[cc]:jb_of[cc] + K],
                    lhsT=xsb[go % NXS][:, (cc % GRP) * D:(cc % GRP + 1) * D],
                    rhs=msk[cc % NM][:],
                    start=False, stop=True, skip_group_check=True,
                ).then_inc(sem_pl, 1)

        with nc.Block() as block:

            @block.sync
            def _(sync):
                sync.dma_start(out=wcol_t[:], in_=wcol_h[:]).then_inc(sem_cst, 16)
                sync.dma_start(out=ident_t[:], in_=ident_h[:]).then_inc(sem_cst, 16)
                sync.dma_start(out=iota_t[:], in_=iota_h[:]).then_inc(sem_cst, 16)
                sync.dma_start(out=bl_t[:], in_=bl_h[:]).then_inc(sem_cst, 16)
                for s in range(nsup):
                    j = s % NXB
                    ch = sup_sizes[s]
                    if s >= NXB:
                        sync.wait_ge(sem_tr, CH0[s - NXB + 1])
                    sync.dma_start(
                        out=xt[j][:, 0:ch * D],
                        in_=xp_h[CH0[s] * P * D:CH0[s + 1] * P * D].rearrange(
                            "(d f) -> d f", d=P),
                    ).then_inc(sem_x[j], 16)
                for b in range(n_banks):
                    sync.wait_ge(sem_fl, b + 1)
                    sync.dma_start(
                        out=out_h[:, b * 512:(b + 1) * 512],
                        in_=stage_t[:, b * 512:(b + 1) * 512],
                    ).then_inc(sem_out, 16)
                sync.wait_ge(sem_zr, 1)
                sync.dma_start(out=z_h[:], in_=zsum_t[:]).then_inc(sem_out, 16)
                sync.wait_ge(sem_out, 16 * (n_banks + 1))

            @block.tensor
            def _(tensor):
                tensor.wait_ge(sem_cst, 16 * N_CST)
                tensor.wait_ge(sem_ini, 2)
                for c in range(nch):
                    s = sup_of[c]
                    ci = c - CH0[s]
                    if ci == 0:
                        tensor.wait_ge(sem_x[s % NXB], 16 * (s // NXB + 1))
                    if c % EXPW == 0 and c // EXPW >= 2:
                        # reuse of this parity's score bank: prior strip's Exp
                        tensor.wait_ge(sem_ex, c // EXPW - 1)
                    xsl = xt[s % NXB][:, ci * D:(ci + 1) * D]
                    bnk, col = sploc(c)
                    nc.tensor.matmul(
                        bnk[:, col:col + 1],
                        lhsT=xsl, rhs=wcol_t[:],
                        start=True, stop=True, skip_group_check=True,
                    ).then_inc(sem_sc, 1)
                    g = c // GRP
                    if c % GRP == 0 and g >= NT:
                        go2 = g - NT
                        tensor.wait_ge(sem_cp[copy_eng[go2]], cp_tick[go2])
                    nc.tensor.transpose(
                        tp[g % NT][:, (c % GRP) * D:(c % GRP + 1) * D],
                        xsl, ident_t[:],
                    ).then_inc(sem_tr, 1)
                    if c % GRP == GRP - 1 and g >= LAG_G:
                        pooled_group(tensor, g - LAG_G, False)
                for go in range(max(0, ngrp - LAG_G), ngrp):
                    pooled_group(tensor, go, True)

            # Copies run at position p; exp/masks trail at p-MLAG so neither
            # ACT nor DVE blocks on exp before emitting a copy PE waits on.

            @block.scalar
            def _(scalar):
                nfl = 0
                for p in range(0, nch + MLAG, GRP):
                    g = p // GRP
                    if g < ngrp and copy_eng[g] == 1:
                        scalar.wait_ge(sem_tr, GRP * g + GRP)
                        if g >= NXS:
                            scalar.wait_ge(sem_pl, GRP * (g - NXS) + GRP)
                        nc.scalar.copy(
                            out=xsb[g % NXS][:], in_=tp[g % NT][:],
                        ).then_inc(sem_cp[1], 1)
                    cm = p - MLAG
                    if cm >= 0 and cm % EXPW == 0:
                        e = cm // EXPW
                        c0, c1 = EXPW * e, min(EXPW * e + EXPW, nch)
                        scalar.wait_ge(sem_sc, c1)
                        bnk, col = sploc(c0)
                        nc.scalar.activation(
                            out=expw_t[:, c0:c1],
                            in_=bnk[:, col:col + (c1 - c0)],
                            func=ACTF.Exp,
                        ).then_inc(sem_ex, 1)
                    while nfl < n_banks and G0[nfl + 1] + 80 <= p:
                        scalar.wait_ge(sem_pl, G0[nfl + 1])
                        nc.scalar.copy(
                            out=stage_t[:, nfl * 512:(nfl + 1) * 512],
                            in_=outp2[nfl % 2][:],
                        ).then_inc(sem_fl, 1)
                        nfl += 1
                while nfl < n_banks:
                    scalar.wait_ge(sem_pl, G0[nfl + 1])
                    nc.scalar.copy(
                        out=stage_t[:, nfl * 512:(nfl + 1) * 512],
                        in_=outp2[nfl % 2][:],
                    ).then_inc(sem_fl, 1)
                    nfl += 1

            @block.vector
            def _(vector):
                for b in range(2):
                    nc.vector.memset(outp2[b][:], 0.0).then_inc(sem_ini, 1)
                vector.wait_ge(sem_cst, 16 * N_CST)
                nms = 2
                for p in range(nch + MLAG):
                    g = p // GRP
                    if p < nch and p % GRP == GRP - 1 and copy_eng[g] == 0:
                        vector.wait_ge(sem_tr, GRP * g + GRP)
                        if g >= NXS:
                            vector.wait_ge(sem_pl, GRP * (g - NXS) + GRP)
                        nc.vector.tensor_copy(
                            out=xsb[g % NXS][:], in_=tp[g % NT][:],
                        ).then_inc(sem_cp[0], 1)
                    while nms < n_banks and G0[nms - 1] + 96 <= p:
                        vector.wait_ge(sem_fl, nms - 1)
                        nc.vector.memset(outp2[nms % 2][:], 0.0).then_inc(sem_ini, 1)
                        nms += 1
                    cm = p - MLAG
                    if cm < 0:
                        continue
                    if cm % EXPW == 0:
                        vector.wait_ge(sem_ex, cm // EXPW + 1)
                        if cm >= NM:
                            vector.wait_ge(sem_pl, cm - NM + 1)
                    if mask_dve[cm]:
                        nc.vector.tensor_scalar(
                            msk[cm % NM][:], iota_t[:],
                            bl_t[:, cm:cm + 1], expw_t[:, cm:cm + 1],
                            ALU.is_equal, ALU.mult,
                        ).then_inc(sem_md, 1)
                # Z = sum over all chunks of expw (pads contribute e^0=1
                # each; host subtracts the pad count)
                vector.wait_ge(sem_ex, n_exp)
                nc.vector.tensor_reduce(
                    out=zsum_t[:], in_=expw_t[:],
                    axis=mybir.AxisListType.X, op=ALU.add,
                ).then_inc(sem_zr, 1)

            @block.gpsimd
            def _(gpsimd):
                # GPSIMD cannot access PSUM: masks only
                gpsimd.wait_ge(sem_cst, 16 * N_CST)
                for cm in range(nch):
                    if cm % EXPW == 0:
                        gpsimd.wait_ge(sem_ex, cm // EXPW + 1)
                        if cm >= NM:
                            gpsimd.wait_ge(sem_pl, cm - NM + 1)
                    if not mask_dve[cm]:
                        nc.gpsimd.tensor_scalar(
                            msk[cm % NM][:], iota_t[:],
                            bl_t[:, cm:cm + 1], expw_t[:, cm:cm + 1],
                            ALU.is_equal, ALU.mult,
                        ).then_inc(sem_mg, 1)

    return nc


def _plan(counts_k, n_banks):
    """Pass-1 chunking for one core: counts_k[j] = node count of local seg j.
    Returns per-group chunk lists [(jf, [(j, off, take), ...]), ...]."""
    groups = []
    nsegs = len(counts_k)
    for gb in range(n_banks):
        glo, ghi = 512 * gb, min(512 * (gb + 1), nsegs)
        chunks = []
        cur_nodes, cur_jf, cur_n = [], None, 0
        for j in range(glo, ghi):
            cnt = int(counts_k[j])
            off = 0
            while cnt > 0:
                if cur_jf is not None and j - cur_jf + 1 > KCAP:
                    chunks.append((cur_jf, cur_nodes))
                    cur_nodes, cur_jf, cur_n = [], None, 0
                if cur_jf is None:
                    cur_jf = j
                take = min(cnt, P - cur_n)
                cur_nodes.append((j, off, take))
                cur_n += take
                off += take
                cnt -= take
                if cur_n == P:
                    chunks.append((cur_jf, cur_nodes))
                    cur_nodes, cur_jf, cur_n = [], None, 0
        if cur_n > 0:
            chunks.append((cur_jf, cur_nodes))
        groups.append(chunks)
    return groups


def _pool(x, batch, W, num_graphs, n_cores=N_CORES):
    segs_per_core = num_graphs // n_cores
    n_banks = (segs_per_core + 511) // 512

    counts = np.bincount(batch, minlength=num_graphs).astype(np.int64)
    order = np.argsort(-counts, kind="stable")      # global seg ids, size desc
    orig_starts = np.zeros(num_graphs + 1, np.int64)
    np.cumsum(counts, out=orig_starts[1:])

    # snake deal: sorted position p -> (core, local j)
    nloc = num_graphs // n_cores
    pos = np.arange(num_graphs).reshape(nloc, n_cores)
    core_of_pos = np.where((np.arange(nloc) % 2 == 0)[:, None],
                           np.arange(n_cores)[None, :],
                           np.arange(n_cores)[None, :][:, ::-1])
    local_ids = np.empty((n_cores, nloc), np.int64)
    for k in range(n_cores):
        local_ids[k] = order[pos[core_of_pos == k]]
    local_counts = counts[local_ids]                # [n_cores, nloc]

    plans = [_plan(local_counts[k], n_banks) for k in range(n_cores)]
    ngc = [max(len(plans[k][g]) for k in range(n_cores)) for g in range(n_banks)]
    total = sum(ngc)
    ngc[-1] += (-total) % GRP
    nch = sum(ngc)

    G0 = [0]
    for t in ngc:
        G0.append(G0[-1] + t)
    W0 = np.full(nch, np.iinfo(np.int64).max, np.int64)
    W1 = np.full(nch, -1, np.int64)
    for k in range(n_cores):
        for g in range(n_banks):
            for i, (jf, nodes) in enumerate(plans[k][g]):
                c = G0[g] + i
                W0[c] = min(W0[c], jf)
                W1[c] = max(W1[c], nodes[-1][0])
    bank_of = np.empty(nch, np.int64)
    for g in range(n_banks):
        bank_of[G0[g]:G0[g + 1]] = g
        empt = W1[G0[g]:G0[g + 1]] < 0            # all-core-empty pad chunks
        W0[G0[g]:G0[g + 1]][empt] = 512 * g
        W1[G0[g]:G0[g + 1]][empt] = 512 * g
    K = int(max(2, (W1 - W0).max() + 1))
    jb_of = np.minimum(W0 - 512 * bank_of, 512 - K).astype(np.int64)
    assert jb_of.min() >= 0

    sup_sizes = [SUP_CH] * (nch // SUP_CH)
    if nch % SUP_CH:
        sup_sizes.append(nch % SUP_CH)

    # engine splits (tunable): masks on DVE (frac MD) else GPSIMD;
    # psum->sbuf copies: CPAT cycled over GRP-chunk groups (0=DVE, 1=ACT)
    mfrac = float(os.environ.get("MD", "0.3125"))
    mask_dve = [(int(c * mfrac) != int((c + 1) * mfrac)) for c in range(nch)]
    ngrp = nch // GRP
    cpat = [int(v) for v in os.environ.get("CPAT", "0,1").split(",")]
    copy_eng = [cpat[g % len(cpat)] for g in range(ngrp)]

    # per-core tensors
    x_bf = np.ascontiguousarray(x).astype(ml_dtypes.bfloat16)
    in_maps, pad_counts = [], []
    for k in range(n_cores):
        xflat = np.zeros((nch * P, D), ml_dtypes.bfloat16)
        blflat = np.full((nch * P,), 999.0, np.float32)
        real = 0
        for g in range(n_banks):
            for i, (jf, nodes) in enumerate(plans[k][g]):
                c = G0[g] + i
                base = 512 * bank_of[c] + jb_of[c]
                p0 = c * P
                for (j, off, take) in nodes:
                    gid = local_ids[k][j]
                    s0 = orig_starts[gid] + off
                    xflat[p0:p0 + take] = x_bf[s0:s0 + take]
                    blflat[p0:p0 + take] = j - base
                    p0 += take
                    real += take
        pad_counts.append(nch * P - real)
        # slab per super: (c, n, d) -> (d, c, n)
        slabs = []
        o = 0
        for ch in sup_sizes:
            a = xflat[o * P:(o + ch) * P]
            slabs.append(np.ascontiguousarray(
                a.reshape(ch, P, D).transpose(2, 0, 1)).reshape(-1))
            o += ch
        xp = np.concatenate(slabs)
        bl = np.ascontiguousarray(blflat.reshape(nch, P).T).astype(np.float32)
        in_maps.append({
            "xp": xp, "bl": bl,
            "wcol": np.asarray(W, np.float32).reshape(P, 1).astype(ml_dtypes.bfloat16),
            "ident": np.eye(P, dtype=ml_dtypes.bfloat16),
            "iota": np.broadcast_to(
                np.arange(K).astype(ml_dtypes.bfloat16), (P, K)).copy(),
        })

    key = hashlib.sha1(
        np.concatenate([bank_of, jb_of, [nch, K, n_banks]]).tobytes()
        + bytes(mask_dve) + bytes(copy_eng) + bytes(str(sup_sizes), "ascii")
    ).hexdigest()
    if key not in _prog_cache:
        _prog_cache[key] = _build(nch, K, n_banks, bank_of.tolist(),
                                  jb_of.tolist(), sup_sizes, mask_dve, copy_eng)
    nc = _prog_cache[key]

    res = run_bass_kernel_spmd(nc, in_maps, list(range(n_cores))).results

    z_total = 0.0
    out = np.zeros((num_graphs, D), np.float32)
    for k in range(n_cores):
        z_total += float(res[k]["zout"].astype(np.float64).sum()) - pad_counts[k]
        o = res[k]["outp"].astype(np.float32)       # [D, n_banks*512]
        out[local_ids[k]] = o.T[:nloc]
    return (out / np.float32(z_total)).astype(np.float32)


def kernel(x, batch, W, b):
    x = np.asarray(x, np.float32)
    batch = np.asarray(batch).astype(np.int64)
    W = np.asarray(W, np.float32)
    return _pool(x, batch, W, num_graphs=16384)


if __name__ == "__main__":
    rng = np.random.default_rng(0)
    G = int(os.environ.get("TG", "1024"))
    n = int(os.environ.get("TN", "64000"))
    x = rng.standard_normal((n, D), dtype=np.float32)
    batch = np.sort(rng.integers(0, G, n)).astype(np.int64)
    W = (rng.standard_normal((D, 1), dtype=np.float32) / np.sqrt(D)).astype(np.float32)
    b = np.zeros((1,), np.float32)

    got = _pool(x, batch, W, num_graphs=G)

    s = (x @ W).ravel()
    a = np.exp(s - s.max()); a /= a.sum()
    want = np.zeros((G, D), np.float64)
    np.add.at(want, batch, x * a[:, None])
    want = want.astype(np.float32)
    num = np.abs(got - want).max()
    print("abs err:", num, "rel err:", num / np.abs(want).max())
